# revision 1
# baseline (speedup 1.0000x reference)
"""Trainium2 Bass kernel for nn_AffinityPredictor (2-layer GCN + mean-pool + FC).

Contract: kernel(**inputs) takes the FULL unsharded inputs (as produced by
reference.setup_inputs()) and returns the FULL [1024] output.

Strategy (8 NeuronCores, SPMD — one program, per-core data):
  * Graph-parallel sharding: core c owns graphs [128c, 128(c+1)) and hence a
    contiguous node range (batch is sorted); it owns all edges whose dst falls
    in that range.  Weights + x are replicated.
  * All nodes get a single "unified" row id R (core-major, partition-major
    within a core, padded to 12800 rows/core).  Node features live in HBM
    tables addressed by R — layer 1: dinv * (x @ W1) computed densely on every
    core; layer 2: the layer-1 activations, exchanged with an 8-core
    AllGather.  Tables are stored as bf16 pair-rows [R/2, 128] (two nodes per
    256-byte row) so the MoE bulk-gather primitive (dma_gather, int16 indices,
    256B elements) can fetch per-edge rows; indices are split in two 32k-row
    slabs to satisfy int16.
  * The scatter/segment-sum side is eliminated: the host sorts each core's
    edges by destination into windows of 32 consecutive dst nodes (padded to
    a uniform per-slab tile count so one program serves all cores) and ships
    binary bf16 one-hot tiles (lo/hi pair-half split); the segment sum becomes
    PE matmuls accumulating into a per-window [32, 64] PSUM tile.
  * GCN normalization D^-1/2 (A+I) D^-1/2 is folded in as activation scales
    (dinv at the table build, dinv(dst) — squared for layer 1 — at the window
    flush) and a rank-1 bias matmul with lhsT = sqrt(deg); relu commutes with
    the positive dinv scale.
  * Mean pooling via binary one-hot matmuls; 1/count and the fc bias are
    applied to the final [1, 128] result.

All index/structure preprocessing (degrees, sorting, padding, one-hots)
happens on the host in numpy; every FLOP on x/W data runs on device.
"""

import numpy as np
import ml_dtypes

import concourse.tile as tile
from concourse import bass, bacc, mybir
from concourse.bass_utils import run_bass_kernel_spmd

# ---------------------------------------------------------------- constants
N_NODES = 100_000
NUM_GRAPHS = 1024
IN_DIM = 20
NODE_DIM = 64
N_CORES = 8
GRAPHS_PER_CORE = NUM_GRAPHS // N_CORES      # 128
P = 128

NPAD = 12_800                                # padded nodes per core
NT = NPAD // P                               # node tiles per core = 100
WIN = 32                                     # dst nodes per window
NW = NPAD // WIN                             # windows per core = 400
WPG = P // WIN                               # windows per node tile = 4
NW3 = (NW + 2) // 3

NROWS = N_CORES * NPAD                       # unified rows = 102400
NPAIR = NROWS // 2                           # pair rows = 51200
SLAB = 32_768                                # pair rows per index slab
NSLAB = 2                                    # 51200 -> slabs [32768, 18432]

BF16 = mybir.dt.bfloat16
F32 = mybir.dt.float32
I16 = mybir.dt.int16

_CACHE = {}


# ================================================================ host prep
def _layout_from_tws(tws):
    """Derive the global tile layout from per-(window, slab) tile counts.

    Within each group (node tile = WPG windows): slab0's windows' tiles,
    then slab1's. Returns (tile_base_ws [NW, NSLAB] global tile index of
    each window's first tile, group_base [NT+1] cumulative tiles,
    tiles_gs [NT, NSLAB] tiles per group per slab)."""
    tws = np.asarray(tws, np.int64)
    g_of_w = np.arange(NW) // WPG
    tiles_gs = np.zeros((NT, NSLAB), np.int64)
    for s in range(NSLAB):
        tiles_gs[:, s] = np.bincount(g_of_w, weights=tws[:, s],
                                     minlength=NT).astype(np.int64)
    group_base = np.zeros(NT + 1, np.int64)
    group_base[1:] = np.cumsum(tiles_gs.sum(1))
    tile_base_ws = np.zeros((NW, NSLAB), np.int64)
    for g in range(NT):
        off = group_base[g]
        for s in range(NSLAB):
            for b in range(WPG):
                w = g * WPG + b
                tile_base_ws[w, s] = off
                off += tws[w, s]
    return tile_base_ws, group_base, tiles_gs


def _preprocess(x, edge_index, batch, W1, b1, W2, b2, Wfc, bfc):
    x = np.asarray(x, np.float32)
    edge_index = np.asarray(edge_index, np.int64)
    batch = np.asarray(batch, np.int64)

    n = N_NODES
    loop = np.arange(n, dtype=np.int64)
    src = np.concatenate([edge_index[0], loop])
    dst = np.concatenate([edge_index[1], loop])

    deg = np.bincount(dst, minlength=n).astype(np.float32)
    dinv = np.where(deg > 0, 1.0 / np.sqrt(deg), 0.0).astype(np.float32)
    sdeg = np.where(deg > 0, np.sqrt(deg), 0.0).astype(np.float32)

    gbound = np.searchsorted(batch, np.arange(0, NUM_GRAPHS + 1, GRAPHS_PER_CORE))
    n0s, n1s = gbound[:-1], gbound[1:]

    core_of = np.searchsorted(gbound[1:], np.arange(n), side="right")
    local_of = np.arange(n) - n0s[core_of]
    # unified row id: core-major, partition-major within core
    R_of = core_of * NPAD + (local_of % P) * NT + local_of // P

    # inverse map: unified row r -> node id (or -1 for pad rows)
    node_of_R = np.full(NROWS, -1, np.int64)
    node_of_R[R_of] = np.arange(n)

    # ---- per-core edge partitioning & window packing (slab-split tiles)
    src_pair = R_of[src] // 2
    src_half = (R_of[src] % 2).astype(np.int64)
    src_slab = (src_pair // SLAB).astype(np.int64)

    edst_core = core_of[dst]
    per_core = []
    cnts_all = np.zeros((N_CORES, NW, NSLAB), np.int64)
    for c in range(N_CORES):
        m = edst_core == c
        s_pair, s_half, s_slab, d_c = src_pair[m], src_half[m], src_slab[m], dst[m]
        ld = (d_c - n0s[c]).astype(np.int64)
        # sort so (window, slab) groups are contiguous (pos packing relies on it)
        order = np.lexsort((s_pair, s_slab, ld // WIN))
        s_pair, s_half, s_slab, ld = (s_pair[order], s_half[order],
                                      s_slab[order], ld[order])
        w = ld // WIN
        cnts_all[c] = np.bincount(w * NSLAB + s_slab,
                                  minlength=NW * NSLAB).reshape(NW, NSLAB)
        per_core.append((s_pair, s_half, s_slab, ld))

    # per-(window, slab) tile counts: max over cores (SPMD-uniform structure)
    tws = np.ceil(cnts_all.max(axis=0) / P).astype(np.int64)      # [NW, NSLAB]
    tile_base_ws, group_base, tiles_gs = _layout_from_tws(tws)
    T_TOTAL = int(group_base[-1])
    SLOTS = T_TOTAL * P

    in_maps = []
    for c in range(N_CORES):
        s_pair, s_half, s_slab, ld = per_core[c]
        w = ld // WIN
        # slot layout: group (= node tile = 4 windows) major; within a group:
        # [slab0: per-window tiles][slab1: per-window tiles]; dense pack
        # per (window, slab).
        key = w * NSLAB + s_slab
        cnts_ws = np.bincount(key, minlength=NW * NSLAB)
        starts = np.zeros(NW * NSLAB, np.int64)
        starts[1:] = np.cumsum(cnts_ws)[:-1]
        pos = np.arange(len(ld)) - starts[key]
        slot = tile_base_ws[w, s_slab] * P + pos

        pidx = np.zeros(SLOTS, np.int64)              # pair row within slab
        oneh = np.zeros((SLOTS, 2 * WIN), ml_dtypes.bfloat16)
        pidx[slot] = s_pair - s_slab * SLAB
        oneh[slot, (ld % WIN) + WIN * s_half] = 1.0

        # idx device layout: per (group, slab) contiguous block,
        # 16-partition wrap, replicated to 128 partitions
        idx_dev = np.zeros((P, T_TOTAL * P // 16), np.int16)
        col = 0
        pidx_t = pidx.reshape(T_TOTAL, P)
        for g in range(NT):
            for s in range(NSLAB):
                t0 = int(group_base[g] + (tiles_gs[g, 0] if s else 0))
                blk = pidx_t[t0:t0 + int(tiles_gs[g, s])].reshape(-1)
                nb = blk.size // 16
                idx_dev[:16, col:col + nb] = blk.reshape(nb, 16).T
                col += nb
        idx_dev[16:, :] = np.tile(idx_dev[:16, :], (7, 1))

        oneh = np.ascontiguousarray(
            oneh.reshape(T_TOTAL, P, 2 * WIN).transpose(1, 0, 2)
        ).reshape(P, T_TOTAL * 2 * WIN)

        # ---- per-node scale vectors, local layout (node l = 128 t + p)
        n_real = int(n1s[c] - n0s[c])
        l_arr = np.arange(NPAD)
        gl = np.minimum(n0s[c] + l_arr, n - 1)
        valid = l_arr < n_real
        dinv_l = np.where(valid, dinv[gl], 0.0).astype(np.float32)
        sdeg_l = np.where(valid, sdeg[gl], 0.0).astype(np.float32)
        dinvp = np.ascontiguousarray(dinv_l.reshape(NT, P).T)
        dinv2p = np.ascontiguousarray((dinv_l ** 2).reshape(NT, P).T)
        sdeg3 = np.zeros((65, NW3 * WIN), np.float32)
        sw = sdeg_l.reshape(NW, WIN)
        for r in range(3):
            rows = sw[r::3]
            sdeg3[32 * r, :rows.shape[0] * WIN] = rows.reshape(-1)

        # ---- pooling one-hot (binary)
        cnt_g = np.bincount((batch[n0s[c]:n1s[c]] - c * GRAPHS_PER_CORE).astype(np.int64),
                            minlength=GRAPHS_PER_CORE).astype(np.float32)
        invc = (1.0 / np.maximum(cnt_g, 1.0)).astype(np.float32).reshape(1, GRAPHS_PER_CORE)
        lg = (batch[gl] - c * GRAPHS_PER_CORE).astype(np.int64)
        poolh = np.zeros((NPAD, GRAPHS_PER_CORE), ml_dtypes.bfloat16)
        poolh[l_arr[valid], lg[valid]] = 1.0
        poolh = np.ascontiguousarray(
            poolh.reshape(NT, P, GRAPHS_PER_CORE).transpose(1, 0, 2)
        ).reshape(P, NT * GRAPHS_PER_CORE)

        in_maps.append({
            "gidx": idx_dev, "oneh": oneh, "poolh": poolh,
            "dinvp": dinvp, "dinv2p": dinv2p, "invc": invc,
            "sdegrow": sdeg_l.reshape(1, NPAD).astype(ml_dtypes.bfloat16),
        })

    # ---- replicated tensors: raw dinv-scaled x in unified pair-row order.
    # L1 gathers these 20-dim rows and aggregates BEFORE applying W1
    # (A_hat X W1 == (A_hat X) W1), killing the dense x@W1 table stage.
    xs = (dinv[:, None] * x).astype(ml_dtypes.bfloat16)       # [n, 20]
    xtab = np.zeros((NROWS, NODE_DIM), ml_dtypes.bfloat16)    # [102400, 64]
    vmask = node_of_R >= 0
    xtab[vmask, :IN_DIM] = xs[node_of_R[vmask]]
    xtab = np.ascontiguousarray(xtab.reshape(NPAIR, 2 * NODE_DIM))

    shared = {
        "xtab": xtab,
        "ident": np.eye(P, dtype=ml_dtypes.bfloat16),
        "w1": np.asarray(W1, np.float32).astype(ml_dtypes.bfloat16),
        "b1row": np.asarray(b1, np.float32).reshape(1, NODE_DIM)
            .astype(ml_dtypes.bfloat16),
        "b2row": np.asarray(b2, np.float32).reshape(1, NODE_DIM)
            .astype(ml_dtypes.bfloat16),
        "w2": np.asarray(W2, np.float32).astype(ml_dtypes.bfloat16),
        "wfc": np.asarray(Wfc, np.float32).astype(ml_dtypes.bfloat16),
        "bfc": np.full((1, GRAPHS_PER_CORE), np.float32(np.asarray(bfc).reshape(-1)[0])),
    }
    for m in in_maps:
        m.update(shared)
    return in_maps, tuple(map(tuple, tws.tolist()))


# ============================================================= device program
def _build_program(TPWs, debug=False, stages=5, repeat=1, parts='all', nq=4,
                   l2src='copy'):
    # TPWs: per-(window, slab) tile counts [NW][NSLAB] (tuple of tuples)
    tws = np.asarray(TPWs, np.int64)
    tile_base_ws, group_base, tiles_gs = _layout_from_tws(tws)
    T_TOTAL = int(group_base[-1])
    TPG_g = tiles_gs.sum(1)              # tiles per node-group
    TPGMAX = int(TPG_g.max())
    IDXC = T_TOTAL * P // 16             # idx cols total
    OH_GROUPS = 2

    nc = bacc.Bacc(num_swdge_queues=nq)
    xtab = nc.declare_dram_parameter("xtab", [NPAIR, 2 * NODE_DIM], BF16,
                                     isOutput=False)
    ident = nc.declare_dram_parameter("ident", [P, P], BF16, isOutput=False)
    w1 = nc.declare_dram_parameter("w1", [IN_DIM, NODE_DIM], BF16, isOutput=False)
    b1row = nc.declare_dram_parameter("b1row", [1, NODE_DIM], BF16, isOutput=False)
    b2row = nc.declare_dram_parameter("b2row", [1, NODE_DIM], BF16, isOutput=False)
    sdegrow = nc.declare_dram_parameter("sdegrow", [1, NPAD], BF16, isOutput=False)
    w2 = nc.declare_dram_parameter("w2", [NODE_DIM, NODE_DIM], BF16, isOutput=False)
    wfc = nc.declare_dram_parameter("wfc", [NODE_DIM, 1], BF16, isOutput=False)
    bfc = nc.declare_dram_parameter("bfc", [1, GRAPHS_PER_CORE], F32, isOutput=False)
    invc = nc.declare_dram_parameter("invc", [1, GRAPHS_PER_CORE], F32, isOutput=False)
    dinvp = nc.declare_dram_parameter("dinvp", [P, NT], F32, isOutput=False)
    dinv2p = nc.declare_dram_parameter("dinv2p", [P, NT], F32, isOutput=False)
    gidx = nc.declare_dram_parameter("gidx", [P, IDXC], I16, isOutput=False)
    oneh = nc.declare_dram_parameter("oneh", [P, T_TOTAL * 2 * WIN], BF16, isOutput=False)
    poolh = nc.declare_dram_parameter("poolh", [P, NT * GRAPHS_PER_CORE], BF16, isOutput=False)
    out = nc.declare_dram_parameter("out", [1, GRAPHS_PER_CORE], F32, isOutput=True)

    h1_slice = nc.dram_tensor("h1_slice", [NPAD, NODE_DIM], BF16)
    table2 = nc.dram_tensor("table2", [NROWS, NODE_DIM], BF16, addr_space="Shared")
    table2l = nc.dram_tensor("table2l", [NROWS, NODE_DIM], BF16)
    xt_pair = xtab[:]                                                # [51200, 128]
    t2_src = table2l if l2src == 'copy' else table2
    t2_pair = t2_src[:].rearrange("(q two) d -> q (two d)", two=2)   # [51200, 128]

    if debug:
        dbg_h1 = nc.declare_dram_parameter("dbg_h1", [P, NT * NODE_DIM], BF16, isOutput=True)
        dbg_h2 = nc.declare_dram_parameter("dbg_h2", [P, NT * NODE_DIM], BF16, isOutput=True)
        dbg_gt = nc.declare_dram_parameter("dbg_gt", [P, TPGMAX * 2 * NODE_DIM], BF16, isOutput=True)
        dbg_ps = nc.declare_dram_parameter("dbg_ps", [32, NODE_DIM], F32, isOutput=True)
        dbg_pool = nc.declare_dram_parameter("dbg_pool", [NODE_DIM, GRAPHS_PER_CORE], F32, isOutput=True)

    with tile.TileContext(nc) as tc:
        with (
            tc.tile_pool(name="const", bufs=1) as constp,
            tc.tile_pool(name="idxp", bufs=3) as idxp,
            tc.tile_pool(name="stage", bufs=2) as stagep,
            tc.tile_pool(name="gat", bufs=7) as gatp,
            tc.tile_pool(name="ohp", bufs=2) as ohp,
            tc.tile_pool(name="hsb", bufs=1) as hsbp,
            tc.tile_pool(name="php", bufs=2) as php,
            tc.tile_pool(name="psAg", bufs=1, space="PSUM") as psAg,
            tc.tile_pool(name="psD", bufs=1, space="PSUM") as psD,
            tc.tile_pool(name="psB", bufs=3, space="PSUM") as psB,
            tc.tile_pool(name="psT", bufs=1, space="PSUM") as psT,
            tc.tile_pool(name="psC", bufs=1, space="PSUM") as psC,
        ):
            # ---------------- constants
            w1_sb = constp.tile([IN_DIM, NODE_DIM], BF16)
            b1row_sb = constp.tile([1, NODE_DIM], BF16)
            b2row_sb = constp.tile([1, NODE_DIM], BF16)
            sdegrow_sb = constp.tile([1, NPAD], BF16)
            w2_sb = constp.tile([NODE_DIM, NODE_DIM], BF16)
            wfc_sb = constp.tile([NODE_DIM, 1], BF16)
            bfc_sb = constp.tile([1, GRAPHS_PER_CORE], F32)
            invc_sb = constp.tile([1, GRAPHS_PER_CORE], F32)
            dinv_sb = constp.tile([P, NT], F32)
            dinv2_sb = constp.tile([P, NT], F32)
            id_sb = constp.tile([P, P], BF16)
            nc.sync.dma_start(out=id_sb[:], in_=ident[:])
            for dst_t, src_t in ((w1_sb, w1), (b1row_sb, b1row),
                                 (b2row_sb, b2row), (sdegrow_sb, sdegrow),
                                 (w2_sb, w2), (wfc_sb, wfc), (bfc_sb, bfc),
                                 (invc_sb, invc), (dinv_sb, dinvp),
                                 (dinv2_sb, dinv2p)):
                nc.sync.dma_start(out=dst_t[:], in_=src_t[:])

            # ---------------- message-passing layers
            def grp_tlist(bg, b_):
                # within-group tile indices serving window (bg, b_)
                w = bg * WPG + b_
                base0 = int(tile_base_ws[w, 0] - group_base[bg])
                lst = list(range(base0, base0 + int(tws[w, 0])))
                if NSLAB > 1:
                    base1 = int(tile_base_ws[w, 1] - group_base[bg])
                    lst += list(range(base1, base1 + int(tws[w, 1])))
                return lst

            h_sb = {}
            # mean-pool PSUM accumulator; matmuls are folded into the L2
            # flush loop so pooling hides behind the L2 gathers
            pool_ps = psC.tile([NODE_DIM, GRAPHS_PER_CORE], F32, tag="pps")
            PHC = 25
            ph_sb = None
            layer_list = [l for l in (1, 2) if stages >= (2 if l == 1 else 4)] * repeat
            for layer in layer_list:
                table_p = xt_pair if (layer == 1 or l2src == 'xtab') else t2_pair
                scale_sb = dinv2_sb if layer == 1 else dinv_sb
                h = hsbp.tile([P, NT * NODE_DIM], BF16, tag=f"h{layer}")
                h_sb[layer] = h

                if parts == 'gather_only':
                    nc.vector.memset(h[:], 0.0)
                static_gt = None
                if parts == 'mm_only':
                    static_gt = gatp.tile([P, TPGMAX * 2 * NODE_DIM], BF16, tag="gt")
                    nc.vector.memset(static_gt[:], 0.0)
                for og in range(0, NT, OH_GROUPS):
                    ogn = min(OH_GROUPS, NT - og)
                    tb0, tb1 = int(group_base[og]), int(group_base[og + ogn])
                    ntog = tb1 - tb0
                    oh_sb = ix_sb = None
                    if ntog > 0:
                        oh_sb = ohp.tile([P, OH_GROUPS * TPGMAX * 2 * WIN], BF16,
                                         tag="oh")
                        nc.scalar.dma_start(
                            out=oh_sb[:, :ntog * 2 * WIN],
                            in_=oneh[:, tb0 * 2 * WIN:tb1 * 2 * WIN],
                        )
                        ix_sb = idxp.tile([P, OH_GROUPS * TPGMAX * 8], I16, tag="ix")
                        nc.sync.dma_start(
                            out=ix_sb[:, :ntog * 8],
                            in_=gidx[:, tb0 * 8:tb1 * 8],
                        )
                    for bg in range(og, og + ogn):
                        # gather: one call per slab; tile layout within group:
                        # [slab0: per-window tiles][slab1: per-window tiles]
                        TPGb = int(TPG_g[bg])
                        do_mm = parts in ('all', 'mm_only')
                        if TPGb == 0:
                            # padding-only group: no edges on any core
                            if do_mm:
                                nc.vector.memset(
                                    h[:, bg * NODE_DIM:(bg + 1) * NODE_DIM], 0.0)
                        gt = gtv = None
                        if parts == 'mm_only':
                            gt = static_gt
                        elif TPGb > 0:
                            gt = gatp.tile([P, TPGMAX * 2 * NODE_DIM], BF16,
                                           tag="gt")
                        if TPGb > 0:
                            gtv = gt[:, :TPGb * 2 * NODE_DIM].rearrange(
                                "p (t r) -> p t r", r=2 * NODE_DIM)
                        colb = (int(group_base[bg]) - tb0) * 8
                        tile0 = 0
                        for s in (range(NSLAB)
                                  if TPGb > 0 and parts in ('all', 'gather_only')
                                  else []):
                            nts = int(tiles_gs[bg, s])
                            if nts == 0:
                                continue
                            nc.gpsimd.dma_gather(
                                out_ap=gtv[:, tile0:tile0 + nts, :],
                                in_ap=table_p[s * SLAB:
                                              min((s + 1) * SLAB, table_p.shape[0]), :],
                                idxs_ap=ix_sb[:, colb:colb + nts * 8],
                                num_idxs=nts * P,
                                num_idxs_reg=nts * P,
                                elem_size=2 * NODE_DIM,
                                single_packet=False,
                                queue_num=(bg + s) % nq,
                            )
                            colb += nts * 8
                            tile0 += nts
                        if layer == 1 and do_mm and TPGb > 0:
                            # transposed aggregation: ps_agg[:, dst] sums the
                            # 20-dim dinv-scaled x rows; W1 applied after.
                            ps_agg = psAg.tile([IN_DIM, P], F32, tag="agg")
                            for b_ in range(WPG):
                                tlist = grp_tlist(bg, b_)
                                for ti, t in enumerate(tlist):
                                    ohb = (int(group_base[bg]) - tb0 + t) * 2 * WIN
                                    for hf in range(2):
                                        nc.tensor.matmul(
                                            out=ps_agg[:, b_ * WIN:(b_ + 1) * WIN],
                                            lhsT=gt[:, t * 2 * NODE_DIM + hf * NODE_DIM:
                                                   t * 2 * NODE_DIM + hf * NODE_DIM + IN_DIM],
                                            rhs=oh_sb[:, ohb + hf * WIN:ohb + (hf + 1) * WIN],
                                            start=(ti == 0 and hf == 0),
                                            stop=(ti == len(tlist) - 1 and hf == 1),
                                        )
                            aggT = stagep.tile([IN_DIM, P], BF16, tag="aggT")
                            nc.vector.tensor_copy(out=aggT[:], in_=ps_agg[:])
                            ps2 = psD.tile([P, NODE_DIM], F32, tag="ps2")
                            nc.tensor.matmul(out=ps2[:], lhsT=aggT[:], rhs=w1_sb[:],
                                             start=True, stop=False)
                            nc.tensor.matmul(
                                out=ps2[:], lhsT=sdegrow_sb[:, bg * P:(bg + 1) * P],
                                rhs=b1row_sb[:], start=False, stop=True)
                            nc.scalar.activation(
                                out=h[:, bg * NODE_DIM:(bg + 1) * NODE_DIM],
                                in_=ps2[:], func=mybir.ActivationFunctionType.Relu,
                                scale=scale_sb[:, bg:bg + 1],
                            )
                        if layer == 2 and do_mm and TPGb > 0:
                            # transposed aggregation [64 feats, 128 dsts],
                            # then one PE transpose + rank-1 bias per tile
                            ps_a2 = psB.tile([NODE_DIM, P], F32, tag="agg2")
                            for b_ in range(WPG):
                                tlist = grp_tlist(bg, b_)
                                for ti, t in enumerate(tlist):
                                    ohb = (int(group_base[bg]) - tb0 + t) * 2 * WIN
                                    for hf in range(2):
                                        nc.tensor.matmul(
                                            out=ps_a2[:, b_ * WIN:(b_ + 1) * WIN],
                                            lhsT=gt[:, t * 2 * NODE_DIM + hf * NODE_DIM:
                                                   t * 2 * NODE_DIM + (hf + 1) * NODE_DIM],
                                            rhs=oh_sb[:, ohb + hf * WIN:ohb + (hf + 1) * WIN],
                                            start=(ti == 0 and hf == 0),
                                            stop=(ti == len(tlist) - 1 and hf == 1),
                                        )
                            h2T = stagep.tile([NODE_DIM, P], BF16, tag="h2T")
                            nc.vector.tensor_copy(out=h2T[:], in_=ps_a2[:])
                            ps3 = psD.tile([P, NODE_DIM], F32, tag="ps2")
                            nc.tensor.matmul(out=ps3[:], lhsT=h2T[:],
                                             rhs=id_sb[:NODE_DIM, :NODE_DIM],
                                             start=True, stop=False)
                            nc.tensor.matmul(
                                out=ps3[:], lhsT=sdegrow_sb[:, bg * P:(bg + 1) * P],
                                rhs=b2row_sb[:], start=False, stop=True)
                            nc.scalar.activation(
                                out=h[:, bg * NODE_DIM:(bg + 1) * NODE_DIM],
                                in_=ps3[:], func=mybir.ActivationFunctionType.Relu,
                                scale=scale_sb[:, bg:bg + 1],
                            )
                        if layer == 2 and stages >= 5 and parts == 'all':
                            if bg % PHC == 0:
                                ph_sb = php.tile([P, PHC * GRAPHS_PER_CORE], BF16,
                                                 tag="ph")
                                nc.scalar.dma_start(
                                    out=ph_sb[:],
                                    in_=poolh[:, bg * GRAPHS_PER_CORE:
                                              (bg + PHC) * GRAPHS_PER_CORE])
                            nc.tensor.matmul(
                                out=pool_ps[:],
                                lhsT=h[:, bg * NODE_DIM:(bg + 1) * NODE_DIM],
                                rhs=ph_sb[:, (bg % PHC) * GRAPHS_PER_CORE:
                                          (bg % PHC + 1) * GRAPHS_PER_CORE],
                                start=(bg == 0), stop=(bg == NT - 1),
                            )
                        if debug and layer == 1 and bg == 0:
                            nc.sync.dma_start(out=dbg_gt[:], in_=gt[:])

                if layer == 1 and stages >= 3:
                    # table2 rows must be dinv*(h1 @ W2): conv2 = (A_hat h1) W2
                    # commutes, so transform the slice before the AllGather.
                    # Per node-group: transpose -> W2 matmul -> transpose back.
                    h1w = hsbp.tile([P, NT * NODE_DIM], BF16, tag="h1w")
                    for g in range(NT):
                        psT1 = psT.tile([NODE_DIM, P], BF16, tag="tr")
                        nc.tensor.transpose(
                            out=psT1[:], in_=h[:, g * NODE_DIM:(g + 1) * NODE_DIM],
                            identity=id_sb[:])
                        hT = stagep.tile([NODE_DIM, P], BF16, tag="hT")
                        nc.vector.tensor_copy(out=hT[:], in_=psT1[:])
                        psT2 = psT.tile([NODE_DIM, P], F32, tag="tr")
                        nc.tensor.matmul(out=psT2[:], lhsT=w2_sb[:], rhs=hT[:],
                                         start=True, stop=True)
                        hwT = stagep.tile([NODE_DIM, P], BF16, tag="hwT")
                        nc.vector.tensor_copy(out=hwT[:], in_=psT2[:])
                        psT3 = psT.tile([P, NODE_DIM], BF16, tag="tr")
                        nc.tensor.transpose(
                            out=psT3[:], in_=hwT[:], identity=id_sb[:NODE_DIM, :NODE_DIM])
                        nc.scalar.copy(
                            out=h1w[:, g * NODE_DIM:(g + 1) * NODE_DIM], in_=psT3[:])
                    nc.sync.dma_start(
                        out=h1_slice[:].rearrange("(p t) d -> p (t d)", p=P),
                        in_=h1w[:])
                    if l2src == 'dma':
                        # timing-only: fill table2 with local copies (no AG)
                        for cc in range(N_CORES):
                            nc.sync.dma_start(
                                out=table2[cc * NPAD:(cc + 1) * NPAD, :]
                                    .rearrange("(p t) d -> p (t d)", p=P),
                                in_=h1w[:])
                    else:
                        nc.gpsimd.collective_compute(
                            "AllGather",
                            mybir.AluOpType.bypass,
                            replica_groups=[list(range(N_CORES))],
                            ins=[h1_slice[:]],
                            outs=[table2[:]],
                        )
                    if l2src == 'copy':
                        # Shared-space gathers are ~2x slower; bounce the
                        # gathered table to a regular dram tensor first.
                        t2v = table2[:].rearrange("(p t) d -> p (t d)", p=P)
                        t2lv = table2l[:].rearrange("(p t) d -> p (t d)", p=P)
                        nc.sync.dma_start(out=t2lv[:], in_=t2v[:])
                    tc.strict_bb_all_engine_barrier()
                    if debug:
                        nc.sync.dma_start(out=dbg_h1[:], in_=h[:])
                if layer == 2 and debug:
                    nc.sync.dma_start(out=dbg_h2[:], in_=h[:])

            # ---------------- mean pool + fc (matmuls folded into L2 above)
            if stages < 5:
                zo = stagep.tile([1, GRAPHS_PER_CORE], F32, tag="osb")
                nc.vector.memset(zo[:], 0.0)
                nc.sync.dma_start(out=out[:], in_=zo[:])
            pool_sb = stagep.tile([NODE_DIM, GRAPHS_PER_CORE], BF16, tag="pool")
            if stages >= 5:
                nc.vector.tensor_copy(out=pool_sb[:], in_=pool_ps[:])
            if debug:
                pool_f32 = stagep.tile([NODE_DIM, GRAPHS_PER_CORE], F32, tag="poolf")
                nc.vector.tensor_copy(out=pool_f32[:], in_=pool_ps[:])
                nc.sync.dma_start(out=dbg_pool[:], in_=pool_f32[:])

            if stages >= 5:
                fc_ps = psC.tile([1, GRAPHS_PER_CORE], F32, tag="fc")
                nc.tensor.matmul(out=fc_ps[:], lhsT=wfc_sb[:], rhs=pool_sb[:],
                                 start=True, stop=True)
                out_sb = stagep.tile([1, GRAPHS_PER_CORE], F32, tag="osb")
                nc.vector.tensor_tensor(out=out_sb[:], in0=fc_ps[:], in1=invc_sb[:],
                                        op=mybir.AluOpType.mult)
                nc.vector.tensor_tensor(out=out_sb[:], in0=out_sb[:], in1=bfc_sb[:],
                                        op=mybir.AluOpType.add)
                nc.sync.dma_start(out=out[:], in_=out_sb[:])

    nc.compile()
    return nc


# ================================================================== kernel
def kernel(**inputs) -> np.ndarray:
    in_maps, TPWs = _preprocess(
        inputs["x"], inputs["edge_index"], inputs["batch"],
        inputs["W1"], inputs["b1"], inputs["W2"], inputs["b2"],
        inputs["Wfc"], inputs["bfc"],
    )
    if TPWs not in _CACHE:
        _CACHE[TPWs] = _build_program(TPWs)
    nc = _CACHE[TPWs]
    res = run_bass_kernel_spmd(nc, in_maps, list(range(N_CORES)))
    outs = [res.results[c]["out"].reshape(-1) for c in range(N_CORES)]
    return np.concatenate(outs).astype(np.float32)



# revision 36
# speedup vs baseline: 1.4009x; 1.4009x over previous
"""Trainium2 Bass kernel for nn_AffinityPredictor (2-layer GCN + mean-pool + FC).

Contract: kernel(**inputs) takes the FULL unsharded inputs (as produced by
reference.setup_inputs()) and returns the FULL [1024] output.

v2 design (8 NeuronCores, SPMD — one program, per-core data):
  * Graph-parallel sharding: core c owns graphs [128c, 128(c+1)) and the
    contiguous node range they span (batch is sorted); it owns all non-self
    edges whose dst lies in that range.  Self-loops are applied densely.
  * Nodes are padded to NPAD=12800 per core = NT=100 tiles of P=128; local id
    l = t*128 + p.  Feature tables are stored as bf16 pair rows (two nodes
    per 256-byte row) for the swdge bulk-gather (int16 indices, 256B elems).
    Each core's nodes split into half A (l < 6400) and half B so pair-row ids
    fit int16 (25600 rows per half) — the halves double as the two slabs of
    the per-group gather segments AND as the two AllGather chunks.
  * Per (dst-tile g, half s) the incident edges are packed into 128-slot
    tiles with pads only at the segment tail; the gather's runtime
    num_idxs_reg (per-core counts input) trims pad descriptors entirely.
  * The segment-sum is PE matmuls: per edge-tile, two matmuls (one per
    pair-half) against a [128, 128]-wide one-hot block that routes each slot
    to its dst column.  One-hots are GENERATED ON DEVICE (DVE is_equal of a
    per-slot code word against an iota row) instead of streamed from HBM.
  * GCN norm D^-1/2(A+I)D^-1/2 folds in as activation scales: tables carry
    dinv(src); L1's flush scale is dinv^2(dst) which bakes table2 rows
    dinv*(relu(conv1) @ W2) so L2 needs only dinv(dst); rank-1 sqrt(deg)
    bias matmuls keep b1/b2 exact under the scales.
  * L1's per-group transform (transpose - W2 - transpose) is interleaved
    with the group loop; the AllGather runs in two halves (after group 49
    and 99) so exchange overlaps the L1 tail.  Gathered Shared tables are
    bounced to regular DRAM (Shared-space gathers are slower).
  * Mean pooling via binary one-hot matmuls folded into the L2 loop; 1/count
    and the fc bias applied to the final [1, 128] result.
"""

import os
import numpy as np
import ml_dtypes

import concourse.tile as tile
from concourse import bass, bacc, mybir
from concourse.bass import broadcast_tensor_aps
from concourse.bass_utils import run_bass_kernel_spmd

# ---------------------------------------------------------------- constants
N_NODES = 100_000
NUM_GRAPHS = 1024
IN_DIM = 20
NODE_DIM = 64
N_CORES = 8
GPC = NUM_GRAPHS // N_CORES                  # graphs per core = 128
P = 128

NPAD = 12_800                                # padded nodes per core
NT = NPAD // P                               # node tiles (= groups) = 100
NTH = NT // 2                                # tiles per half = 50
HALF = NPAD // 2                             # nodes per half = 6400
QH = N_CORES * HALF // 2                     # pair rows per half-table = 25600
NSEG = NT * 2                                # gather segments (g, s)

PAD_CODE = 300.0                             # one-hot code for pad slots

BF16 = mybir.dt.bfloat16
F32 = mybir.dt.float32
I16 = mybir.dt.int16
I32 = mybir.dt.int32

_CACHE = {}


# ================================================================ host prep
def _preprocess(x, edge_index, batch, W1, b1, W2, b2, Wfc, bfc):
    x = np.asarray(x, np.float32)
    edge_index = np.asarray(edge_index, np.int64)
    batch = np.asarray(batch, np.int64)
    n = N_NODES

    deg = (np.bincount(edge_index[1], minlength=n) + 1).astype(np.float32)
    dinv = (1.0 / np.sqrt(deg)).astype(np.float32)
    sdeg = np.sqrt(deg).astype(np.float32)

    gbound = np.searchsorted(batch, np.arange(0, NUM_GRAPHS + 1, GPC))
    n0s, n1s = gbound[:-1], gbound[1:]
    core_of = np.searchsorted(gbound[1:], np.arange(n), side="right")
    local_of = np.arange(n) - n0s[core_of]           # l = t*128 + p

    # ---- per-edge fields (dst-core partition, self-loops excluded)
    e_src, e_dst = edge_index[0], edge_index[1]
    ecore = core_of[e_dst]
    l_s = local_of[e_src]
    s_half = (l_s >= HALF).astype(np.int64)
    q_all = core_of[e_src] * (HALF // 2) + np.where(s_half == 0, l_s, l_s - HALF) // 2
    hf_all = (l_s % 2).astype(np.int64)

    cnts = np.zeros((N_CORES, NT, 2), np.int64)
    per_core = []
    for c in range(N_CORES):
        m = ecore == c
        ld = (e_dst[m] - n0s[c]).astype(np.int64)
        g = ld // P
        key = g * 2 + s_half[m]
        order = np.argsort(key, kind="stable")
        per_core.append((q_all[m][order], hf_all[m][order],
                         (ld % P)[order], key[order]))
        cnts[c] = np.bincount(key, minlength=NSEG).reshape(NT, 2)

    ts = np.ceil(cnts.max(axis=0) / P).astype(np.int64)          # [NT, 2]
    # segments where some core has 0 edges but ts>0 need a dummy descriptor
    need_dummy = (ts > 0)
    seg_tiles = ts.reshape(-1)                                   # [NSEG]
    seg_base = np.zeros(NSEG + 1, np.int64)
    seg_base[1:] = np.cumsum(seg_tiles)
    T_TOTAL = int(seg_base[-1])

    in_maps = []
    for c in range(N_CORES):
        q, hf, d, key = per_core[c]
        cnt = np.bincount(key, minlength=NSEG)
        starts = np.zeros(NSEG, np.int64)
        starts[1:] = np.cumsum(cnt)[:-1]
        pos = np.arange(len(q)) - starts[key]
        slot = seg_base[key] * P + pos                           # global slot

        idx_flat = np.full(T_TOTAL * P, -1, np.int16)
        wv_flat = np.full(T_TOTAL * P, PAD_CODE, np.float32)
        idx_flat[slot] = q.astype(np.int16)
        wv_flat[slot] = (hf * P + d).astype(np.float32)
        gcnt = cnt.astype(np.int32).copy()
        # dummy valid descriptor for empty segments with ts>0
        empty = (cnt == 0) & need_dummy.reshape(-1)
        for k in np.nonzero(empty)[0]:
            idx_flat[seg_base[k] * P] = 0
            gcnt[k] = 1
        if os.environ.get("PAD_FULL"):
            # probe mode: no runtime trimming — pads fetch row 0
            idx_flat[idx_flat < 0] = 0
            gcnt = (seg_tiles * P).astype(np.int32)

        # idx device layout: 16-partition wrap, element i at [i%16, i//16],
        # replicated to all 8 gpsimd cores' partition groups
        idx_dev = np.tile(np.ascontiguousarray(
            idx_flat.reshape(T_TOTAL * P // 16, 16).T), (8, 1))
        # wvals: slot i of tile t -> [i%128, t]
        wv_dev = np.ascontiguousarray(
            wv_flat.reshape(T_TOTAL, P).T).astype(ml_dtypes.bfloat16)

        # ---- per-node data, local layout (node l = 128 t + p)
        n_real = int(n1s[c] - n0s[c])
        l_arr = np.arange(NPAD)
        gl = np.minimum(n0s[c] + l_arr, n - 1)
        valid = l_arr < n_real
        dinv_l = np.where(valid, dinv[gl], 0.0).astype(np.float32)
        sdeg_l = np.where(valid, sdeg[gl], 0.0).astype(np.float32)
        dinvp = np.ascontiguousarray(dinv_l.reshape(NT, P).T)
        dinv2p = np.ascontiguousarray((dinv_l ** 2).reshape(NT, P).T)

        xs_l = np.zeros((NPAD, IN_DIM), np.float32)
        xs_l[valid] = dinv[gl[valid], None] * x[gl[valid]]
        # xr: [p, t*20+j] = xs_l[t*128+p, j]
        xr = np.ascontiguousarray(
            xs_l.reshape(NT, P, IN_DIM).transpose(1, 0, 2)).reshape(P, NT * IN_DIM)

        # ---- pooling one-hot (binary) + 1/count
        cnt_g = np.bincount((batch[n0s[c]:n1s[c]] - c * GPC).astype(np.int64),
                            minlength=GPC).astype(np.float32)
        invc = (1.0 / np.maximum(cnt_g, 1.0)).astype(np.float32).reshape(1, GPC)
        lg = (batch[gl] - c * GPC).astype(np.int64)
        poolh = np.zeros((NPAD, GPC), ml_dtypes.bfloat16)
        poolh[l_arr[valid], lg[valid]] = 1.0
        poolh = np.ascontiguousarray(
            poolh.reshape(NT, P, GPC).transpose(1, 0, 2)).reshape(P, NT * GPC)

        in_maps.append({
            "gidx": idx_dev, "wvals": wv_dev, "gcnt": gcnt.reshape(1, NSEG),
            "xr": xr.astype(ml_dtypes.bfloat16),
            "dinvp": dinvp, "dinv2p": dinv2p, "invc": invc, "poolh": poolh,
            "sdegrow": sdeg_l.reshape(1, NPAD).astype(ml_dtypes.bfloat16),
        })

    # ---- replicated tables: x*dinv in half-A/B pair-row order
    # half A rows: node (c, l<6400) at pair qa = c*3200 + l//2, col hf*64+j
    xs = dinv[:, None] * x                                        # [n, 20]
    xtab = np.zeros((2, QH, 2 * NODE_DIM), np.float32)
    nodes = np.arange(n)
    l_n = local_of[nodes]
    s_n = (l_n >= HALF).astype(np.int64)
    q_n = core_of[nodes] * (HALF // 2) + np.where(s_n == 0, l_n, l_n - HALF) // 2
    hf_n = l_n % 2
    for j in range(IN_DIM):
        xtab[s_n, q_n, hf_n * NODE_DIM + j] = xs[nodes, j]
    xtabA = xtab[0].astype(ml_dtypes.bfloat16)
    xtabB = xtab[1].astype(ml_dtypes.bfloat16)

    iota = np.tile(np.arange(256, dtype=np.float32), (P, 1)).astype(
        ml_dtypes.bfloat16)

    shared = {
        "xtabA": xtabA, "xtabB": xtabB, "iota": iota,
        "ident": np.eye(P, dtype=ml_dtypes.bfloat16),
        "w1": np.asarray(W1, np.float32).astype(ml_dtypes.bfloat16),
        "b1row": np.asarray(b1, np.float32).reshape(1, NODE_DIM)
            .astype(ml_dtypes.bfloat16),
        "b2row": np.asarray(b2, np.float32).reshape(1, NODE_DIM)
            .astype(ml_dtypes.bfloat16),
        "w2": np.asarray(W2, np.float32).astype(ml_dtypes.bfloat16),
        "wfc": np.asarray(Wfc, np.float32).astype(ml_dtypes.bfloat16),
        "bfc": np.full((1, GPC), np.float32(np.asarray(bfc).reshape(-1)[0])),
    }
    for m in in_maps:
        m.update(shared)
    return in_maps, tuple(map(tuple, ts.tolist()))


# ============================================================= device program
def _build_null(TS):
    """Input-identical no-op program for differential timing."""
    ts = np.asarray(TS, np.int64)
    T_TOTAL = int(ts.sum())
    nc = bacc.Bacc(num_swdge_queues=4)
    nc.declare_dram_parameter("xtabA", [QH, 2 * NODE_DIM], BF16, isOutput=False)
    nc.declare_dram_parameter("xtabB", [QH, 2 * NODE_DIM], BF16, isOutput=False)
    nc.declare_dram_parameter("gidx", [P, T_TOTAL * 8], I16, isOutput=False)
    nc.declare_dram_parameter("wvals", [P, T_TOTAL], BF16, isOutput=False)
    nc.declare_dram_parameter("gcnt", [1, NSEG], I32, isOutput=False)
    nc.declare_dram_parameter("xr", [P, NT * IN_DIM], BF16, isOutput=False)
    nc.declare_dram_parameter("iota", [P, 256], BF16, isOutput=False)
    nc.declare_dram_parameter("ident", [P, P], BF16, isOutput=False)
    nc.declare_dram_parameter("w1", [IN_DIM, NODE_DIM], BF16, isOutput=False)
    nc.declare_dram_parameter("b1row", [1, NODE_DIM], BF16, isOutput=False)
    nc.declare_dram_parameter("b2row", [1, NODE_DIM], BF16, isOutput=False)
    nc.declare_dram_parameter("sdegrow", [1, NPAD], BF16, isOutput=False)
    nc.declare_dram_parameter("w2", [NODE_DIM, NODE_DIM], BF16, isOutput=False)
    nc.declare_dram_parameter("wfc", [NODE_DIM, 1], BF16, isOutput=False)
    nc.declare_dram_parameter("bfc", [1, GPC], F32, isOutput=False)
    nc.declare_dram_parameter("invc", [1, GPC], F32, isOutput=False)
    nc.declare_dram_parameter("dinvp", [P, NT], F32, isOutput=False)
    nc.declare_dram_parameter("dinv2p", [P, NT], F32, isOutput=False)
    nc.declare_dram_parameter("poolh", [P, NT * GPC], BF16, isOutput=False)
    out = nc.declare_dram_parameter("out", [1, GPC], F32, isOutput=True)
    with tile.TileContext(nc) as tc:
        with tc.tile_pool(name="stage", bufs=1) as stagep:
            zo = stagep.tile([1, GPC], F32, tag="osb")
            nc.vector.memset(zo[:], 0.0)
            nc.sync.dma_start(out=out[:], in_=zo[:])
    nc.compile()
    return nc


def _build_program(TS, stages=5, repeat=1, parts='all', nq=4,
                   gt_bufs=5, oh_bufs=2, debug=False, xg='after'):
    if stages == 0:
        return _build_null(TS)
    ts = np.asarray(TS, np.int64)                    # [NT, 2]
    seg_tiles = ts.reshape(-1)
    seg_base = np.zeros(NSEG + 1, np.int64)
    seg_base[1:] = np.cumsum(seg_tiles)
    T_TOTAL = int(seg_base[-1])
    TPG = ts.sum(1)                                  # tiles per group
    TPGMAX = int(TPG.max())
    grp_base = np.zeros(NT + 1, np.int64)
    grp_base[1:] = np.cumsum(TPG)

    nc = bacc.Bacc(num_swdge_queues=nq)
    xtabA = nc.declare_dram_parameter("xtabA", [QH, 2 * NODE_DIM], BF16, isOutput=False)
    xtabB = nc.declare_dram_parameter("xtabB", [QH, 2 * NODE_DIM], BF16, isOutput=False)
    gidx = nc.declare_dram_parameter("gidx", [P, T_TOTAL * 8], I16, isOutput=False)
    wvals = nc.declare_dram_parameter("wvals", [P, T_TOTAL], BF16, isOutput=False)
    gcnt = nc.declare_dram_parameter("gcnt", [1, NSEG], I32, isOutput=False)
    xr = nc.declare_dram_parameter("xr", [P, NT * IN_DIM], BF16, isOutput=False)
    iota = nc.declare_dram_parameter("iota", [P, 256], BF16, isOutput=False)
    ident = nc.declare_dram_parameter("ident", [P, P], BF16, isOutput=False)
    w1 = nc.declare_dram_parameter("w1", [IN_DIM, NODE_DIM], BF16, isOutput=False)
    b1row = nc.declare_dram_parameter("b1row", [1, NODE_DIM], BF16, isOutput=False)
    b2row = nc.declare_dram_parameter("b2row", [1, NODE_DIM], BF16, isOutput=False)
    sdegrow = nc.declare_dram_parameter("sdegrow", [1, NPAD], BF16, isOutput=False)
    w2 = nc.declare_dram_parameter("w2", [NODE_DIM, NODE_DIM], BF16, isOutput=False)
    wfc = nc.declare_dram_parameter("wfc", [NODE_DIM, 1], BF16, isOutput=False)
    bfc = nc.declare_dram_parameter("bfc", [1, GPC], F32, isOutput=False)
    invc = nc.declare_dram_parameter("invc", [1, GPC], F32, isOutput=False)
    dinvp = nc.declare_dram_parameter("dinvp", [P, NT], F32, isOutput=False)
    dinv2p = nc.declare_dram_parameter("dinv2p", [P, NT], F32, isOutput=False)
    poolh = nc.declare_dram_parameter("poolh", [P, NT * GPC], BF16, isOutput=False)
    out = nc.declare_dram_parameter("out", [1, GPC], F32, isOutput=True)
    if debug:
        dbg_gt = nc.declare_dram_parameter("dbg_gt", [P, TPGMAX * P], BF16, isOutput=True)
        dbg_oh = nc.declare_dram_parameter("dbg_oh", [P, TPGMAX * 256], BF16, isOutput=True)
        dbg_agg = nc.declare_dram_parameter("dbg_agg", [NODE_DIM, P], BF16, isOutput=True)
        dbg_h1w = nc.declare_dram_parameter("dbg_h1w", [P, NT * NODE_DIM], BF16, isOutput=True)
        dbg_t2 = nc.declare_dram_parameter("dbg_t2", [P, N_CORES * HALF * NODE_DIM // P], BF16, isOutput=True)
        dbg_pool = nc.declare_dram_parameter("dbg_pool", [NODE_DIM, GPC], BF16, isOutput=True)
        dbg_hg = nc.declare_dram_parameter("dbg_hg", [P, NODE_DIM], BF16, isOutput=True)
        dbg_hwT = nc.declare_dram_parameter("dbg_hwT", [NODE_DIM, P], BF16, isOutput=True)
        dbg_a = nc.declare_dram_parameter("dbg_a", [P, NT * NODE_DIM], BF16, isOutput=True)
        dbg_b = nc.declare_dram_parameter("dbg_b", [P, NT * NODE_DIM], BF16, isOutput=True)

    h1sA = nc.dram_tensor("h1sA", [N_CORES * HALF // N_CORES, NODE_DIM], BF16)
    h1sB = nc.dram_tensor("h1sB", [HALF, NODE_DIM], BF16)
    t2a = nc.dram_tensor("t2a", [N_CORES * HALF, NODE_DIM], BF16, addr_space="Shared")
    t2b = nc.dram_tensor("t2b", [N_CORES * HALF, NODE_DIM], BF16, addr_space="Shared")
    t2al = nc.dram_tensor("t2al", [N_CORES * HALF, NODE_DIM], BF16)
    t2bl = nc.dram_tensor("t2bl", [N_CORES * HALF, NODE_DIM], BF16)

    xtA_pair = xtabA[:]                                        # [25600, 128]
    xtB_pair = xtabB[:]
    t2al_pair = t2al[:].rearrange("(q two) d -> q (two d)", two=2)
    t2bl_pair = t2bl[:].rearrange("(q two) d -> q (two d)", two=2)

    with tile.TileContext(nc) as tc:
        with (
            tc.tile_pool(name="const", bufs=1) as constp,
            tc.tile_pool(name="stage", bufs=2) as stagep,
            tc.tile_pool(name="gat", bufs=gt_bufs) as gatp,
            tc.tile_pool(name="ohp", bufs=oh_bufs) as ohp,
            tc.tile_pool(name="hsb", bufs=1) as hsbp,
            tc.tile_pool(name="hg", bufs=2) as hgp,
            tc.tile_pool(name="php", bufs=2) as php,
            tc.tile_pool(name="psE", bufs=2, space="PSUM") as psE,
            tc.tile_pool(name="psD", bufs=2, space="PSUM") as psD,
            tc.tile_pool(name="psT", bufs=1, space="PSUM") as psT,
            tc.tile_pool(name="psC", bufs=1, space="PSUM") as psC,
        ):
            # ---------------- constants
            w1_sb = constp.tile([IN_DIM, NODE_DIM], BF16)
            b1row_sb = constp.tile([1, NODE_DIM], BF16)
            b2row_sb = constp.tile([1, NODE_DIM], BF16)
            sdeg_sb = constp.tile([1, NPAD], BF16)
            w2_sb = constp.tile([NODE_DIM, NODE_DIM], BF16)
            wfc_sb = constp.tile([NODE_DIM, 1], BF16)
            bfc_sb = constp.tile([1, GPC], F32)
            invc_sb = constp.tile([1, GPC], F32)
            dinv_sb = constp.tile([P, NT], F32)
            dinv2_sb = constp.tile([P, NT], F32)
            id_sb = constp.tile([P, P], BF16)
            io_sb = constp.tile([P, 256], BF16)
            xr_sb = constp.tile([P, NT * IN_DIM], BF16)
            wv_sb = constp.tile([P, T_TOTAL], BF16)
            ix_sb = constp.tile([P, T_TOTAL * 8], I16)
            cnt_sb = constp.tile([1, NSEG], I32)
            for dst_t, src_t in ((w1_sb, w1), (b1row_sb, b1row),
                                 (b2row_sb, b2row), (sdeg_sb, sdegrow),
                                 (w2_sb, w2), (wfc_sb, wfc), (bfc_sb, bfc),
                                 (invc_sb, invc), (dinv_sb, dinvp),
                                 (dinv2_sb, dinv2p), (id_sb, ident),
                                 (io_sb, iota), (xr_sb, xr), (cnt_sb, gcnt)):
                nc.sync.dma_start(out=dst_t[:], in_=src_t[:])
            nc.scalar.dma_start(out=wv_sb[:], in_=wvals[:])
            nc.scalar.dma_start(out=ix_sb[:], in_=gidx[:])

            h1w = hsbp.tile([P, NT * NODE_DIM], BF16, tag="h1w")
            pool_ps = psC.tile([NODE_DIM, GPC], F32, tag="pps")
            PHC = 25
            ph_sb = None
            cnt_reg = nc.gpsimd.alloc_register("gcnt_reg")

            static_gt = None
            if parts == 'mm_only':
                static_gt = gatp.tile([P, TPGMAX * P], BF16, tag="gt")
                nc.vector.memset(static_gt[:], 0.0)

            # memset the gather buffers once so stale SBUF can't be NaN
            if parts != 'mm_only':
                for _ in range(gt_bufs):
                    z = gatp.tile([P, TPGMAX * P], BF16, tag="gt")
                    nc.vector.memset(z[:], 0.0)

            layer_list = [l for l in (1, 2)] * repeat
            do_mm = parts in ('all', 'mm_only')
            do_gather = parts in ('all', 'gather_only')

            for layer in layer_list:
                tabs = (xtA_pair, xtB_pair) if layer == 1 else (t2al_pair, t2bl_pair)
                scale_sb = dinv2_sb if layer == 1 else dinv_sb
                for g in range(NT):
                    gb = int(grp_base[g])
                    tpg = int(TPG[g])
                    if tpg == 0:
                        continue
                    # ---- one-hot generation for this group's tiles
                    oh = None
                    if do_mm:
                        oh = ohp.tile([P, TPGMAX * 256], BF16, tag="oh")
                        oh3 = oh[:, :tpg * 256].rearrange("p (t j) -> p t j", j=256)
                        b0, b1b = broadcast_tensor_aps(
                            wv_sb[:, gb:gb + tpg].rearrange("p (t o) -> p t o", o=1),
                            io_sb[:].rearrange("p (o j) -> p o j", o=1))
                        nc.vector.tensor_tensor(out=oh3, in0=b0, in1=b1b,
                                                op=mybir.AluOpType.is_equal)
                    # ---- gathers (one per half-segment, runtime counts)
                    if parts == 'mm_only':
                        gt = static_gt
                    else:
                        gt = gatp.tile([P, TPGMAX * P], BF16, tag="gt")
                    if do_gather:
                        t0 = 0
                        for s in range(2):
                            nts = int(ts[g, s])
                            if nts == 0:
                                continue
                            k = g * 2 + s
                            sb = int(seg_base[k])
                            nc.gpsimd.reg_load(cnt_reg, cnt_sb[0:1, k:k + 1])
                            nc.gpsimd.dma_gather(
                                out_ap=gt[:, t0 * P:(t0 + nts) * P].rearrange(
                                    "p (t r) -> p t r", r=P),
                                in_ap=tabs[s][:],
                                idxs_ap=ix_sb[:, sb * 8:(sb + nts) * 8],
                                num_idxs=nts * P,
                                num_idxs_reg=cnt_reg,
                                elem_size=2 * NODE_DIM,
                                single_packet=False,
                                queue_num=k % nq,
                            )
                            t0 += nts
                    if not do_mm:
                        continue
                    if debug and layer == 1 and g == 0:
                        nc.sync.dma_start(out=dbg_gt[:], in_=gt[:])
                        nc.sync.dma_start(out=dbg_oh[:], in_=oh[:])
                    # ---- aggregation matmuls
                    fd = IN_DIM if layer == 1 else NODE_DIM
                    ps = psE.tile([NODE_DIM, P], F32, tag="agg")
                    mms = []
                    for t in range(tpg):
                        for hf in range(2):
                            mms.append((gt[:, t * P + hf * NODE_DIM:
                                           t * P + hf * NODE_DIM + fd],
                                        oh[:, t * 256 + hf * P:
                                           t * 256 + (hf + 1) * P]))
                    # self-loop term
                    if layer == 1:
                        self_lhsT = xr_sb[:, g * IN_DIM:(g + 1) * IN_DIM]
                    else:
                        self_lhsT = h1w[:, g * NODE_DIM:(g + 1) * NODE_DIM]
                    mms.append((self_lhsT, id_sb[:]))
                    for i, (lhsT, rhs) in enumerate(mms):
                        nc.tensor.matmul(out=ps[0:fd, :], lhsT=lhsT, rhs=rhs,
                                         start=(i == 0), stop=(i == len(mms) - 1))
                    # ---- flush
                    aggT = stagep.tile([NODE_DIM, P], BF16, tag="aggT")
                    nc.vector.tensor_copy(out=aggT[0:fd, :], in_=ps[0:fd, :])
                    if debug and layer == 1 and g == 0:
                        nc.sync.dma_start(out=dbg_agg[0:fd, :], in_=aggT[0:fd, :])
                    ps2 = psD.tile([P, NODE_DIM], F32, tag="ps2")
                    if layer == 1:
                        nc.tensor.matmul(out=ps2[:], lhsT=aggT[0:fd, :], rhs=w1_sb[:],
                                         start=True, stop=False)
                        brow = b1row_sb
                    else:
                        nc.tensor.matmul(out=ps2[:], lhsT=aggT[0:fd, :],
                                         rhs=id_sb[:NODE_DIM, :NODE_DIM],
                                         start=True, stop=False)
                        brow = b2row_sb
                    nc.tensor.matmul(out=ps2[:], lhsT=sdeg_sb[:, g * P:(g + 1) * P],
                                     rhs=brow[:], start=False, stop=True)
                    hg = hgp.tile([P, NODE_DIM], BF16, tag="hg")
                    nc.scalar.activation(
                        out=hg[:], in_=ps2[:],
                        func=mybir.ActivationFunctionType.Relu,
                        scale=scale_sb[:, g:g + 1])
                    if debug and layer == 1 and g == 0:
                        nc.sync.dma_start(out=dbg_hg[:], in_=hg[:])

                    if layer == 1:
                        # transform: h1w[:, g] = (relu(...) @ W2) (node-major)
                        psT1 = psT.tile([NODE_DIM, P], BF16, tag="tr")
                        nc.tensor.transpose(out=psT1[:], in_=hg[:], identity=id_sb[:])
                        hT = stagep.tile([NODE_DIM, P], BF16, tag="hT")
                        nc.vector.tensor_copy(out=hT[:], in_=psT1[:])
                        psT2 = psT.tile([NODE_DIM, P], F32, tag="tr")
                        nc.tensor.matmul(out=psT2[:], lhsT=w2_sb[:], rhs=hT[:],
                                         start=True, stop=True)
                        hwT = stagep.tile([NODE_DIM, P], BF16, tag="hwT")
                        nc.vector.tensor_copy(out=hwT[:], in_=psT2[:])
                        if debug and g == 0:
                            nc.sync.dma_start(out=dbg_hwT[:], in_=hwT[:])
                        psT3 = psT.tile([P, NODE_DIM], BF16, tag="tr2")
                        nc.tensor.transpose(out=psT3[:], in_=hwT[:],
                                            identity=id_sb[:NODE_DIM, :NODE_DIM])
                        nc.scalar.copy(out=h1w[:, g * NODE_DIM:(g + 1) * NODE_DIM],
                                       in_=psT3[:])
                        if debug and g == 0:
                            nc.sync.dma_start(out=dbg_hg[:],
                                              in_=h1w[:, 0:NODE_DIM])
                    else:
                        # pooling accumulate
                        if g % PHC == 0:
                            ph_sb = php.tile([P, PHC * GPC], BF16, tag="ph")
                            nc.scalar.dma_start(
                                out=ph_sb[:],
                                in_=poolh[:, g * GPC:(g + PHC) * GPC])
                        nc.tensor.matmul(
                            out=pool_ps[:],
                            lhsT=hg[:],
                            rhs=ph_sb[:, (g % PHC) * GPC:(g % PHC + 1) * GPC],
                            start=(g == 0), stop=(g == NT - 1))

                    # ---- half-table exchange, interleaved with L1 tail
                    if (layer == 1 and g in (NTH - 1, NT - 1) and parts == 'all'
                            and xg == 'inline'):
                        first = g == NTH - 1
                        hsl = h1sA if first else h1sB
                        t2 = t2a if first else t2b
                        t2l = t2al if first else t2bl
                        c0 = 0 if first else NTH * NODE_DIM
                        if debug and first:
                            nc.sync.dma_start(out=dbg_a[:], in_=h1w[:])
                        if debug and not first:
                            nc.sync.dma_start(out=dbg_oh[:, :NT * NODE_DIM],
                                              in_=h1w[:])
                        nc.sync.dma_start(
                            out=hsl[:].rearrange("(t p) d -> p t d", p=P),
                            in_=h1w[:, c0:c0 + NTH * NODE_DIM].rearrange(
                                "p (t d) -> p t d", d=NODE_DIM))
                        nc.gpsimd.collective_compute(
                            "AllGather",
                            mybir.AluOpType.bypass,
                            replica_groups=[list(range(N_CORES))],
                            ins=[hsl[:]],
                            outs=[t2[:]],
                        )
                        nc.sync.dma_start(
                            out=t2l[:].rearrange("(p r) d -> p (r d)", p=P),
                            in_=t2[:].rearrange("(p r) d -> p (r d)", p=P))
                        if debug and first:
                            nc.sync.dma_start(out=dbg_b[:], in_=h1w[:])
                        if not first:
                            tc.strict_bb_all_engine_barrier()
                            if debug:
                                nc.sync.dma_start(out=dbg_h1w[:], in_=h1w[:])
                                nc.sync.dma_start(
                                    out=dbg_t2[:],
                                    in_=t2al[:].rearrange("(p r) d -> p (r d)", p=P))
                                nc.sync.dma_start(
                                    out=dbg_gt[:, :NTH * NODE_DIM],
                                    in_=h1sA[:].rearrange("(t p) d -> p t d", p=P))

                if layer == 1 and parts == 'all' and xg == 'after':
                    for first in (True, False):
                        hsl = h1sA if first else h1sB
                        t2 = t2a if first else t2b
                        c0 = 0 if first else NTH * NODE_DIM
                        nc.sync.dma_start(
                            out=hsl[:].rearrange("(t p) d -> p t d", p=P),
                            in_=h1w[:, c0:c0 + NTH * NODE_DIM].rearrange(
                                "p (t d) -> p t d", d=NODE_DIM))
                        nc.gpsimd.collective_compute(
                            "AllGather",
                            mybir.AluOpType.bypass,
                            replica_groups=[list(range(N_CORES))],
                            ins=[hsl[:]],
                            outs=[t2[:]],
                        )
                    for t2, t2l in ((t2a, t2al), (t2b, t2bl)):
                        nc.sync.dma_start(
                            out=t2l[:].rearrange("(p r) d -> p (r d)", p=P),
                            in_=t2[:].rearrange("(p r) d -> p (r d)", p=P))
                    tc.strict_bb_all_engine_barrier()
                    if debug:
                        nc.sync.dma_start(out=dbg_h1w[:], in_=h1w[:])
                        nc.sync.dma_start(
                            out=dbg_t2[:],
                            in_=t2al[:].rearrange("(p r) d -> p (r d)", p=P))
                if layer == 1 and parts != 'all':
                    # no exchange in attribution modes: fill h1w deterministically
                    nc.vector.memset(h1w[:], 0.0)
                    tc.strict_bb_all_engine_barrier()

            # ---------------- mean pool + fc
            if parts != 'all':
                zo = stagep.tile([1, GPC], F32, tag="osb")
                nc.vector.memset(zo[:], 0.0)
                nc.sync.dma_start(out=out[:], in_=zo[:])
            else:
                pool_sb = stagep.tile([NODE_DIM, GPC], BF16, tag="pool")
                nc.vector.tensor_copy(out=pool_sb[:], in_=pool_ps[:])
                if debug:
                    nc.sync.dma_start(out=dbg_pool[:], in_=pool_sb[:])
                fc_ps = psC.tile([1, GPC], F32, tag="fc")
                nc.tensor.matmul(out=fc_ps[:], lhsT=wfc_sb[:], rhs=pool_sb[:],
                                 start=True, stop=True)
                out_sb = stagep.tile([1, GPC], F32, tag="osb")
                nc.vector.tensor_tensor(out=out_sb[:], in0=fc_ps[:], in1=invc_sb[:],
                                        op=mybir.AluOpType.mult)
                nc.vector.tensor_tensor(out=out_sb[:], in0=out_sb[:], in1=bfc_sb[:],
                                        op=mybir.AluOpType.add)
                nc.sync.dma_start(out=out[:], in_=out_sb[:])

    nc.compile()
    return nc


# ================================================================== kernel
def kernel(**inputs) -> np.ndarray:
    in_maps, TS = _preprocess(
        inputs["x"], inputs["edge_index"], inputs["batch"],
        inputs["W1"], inputs["b1"], inputs["W2"], inputs["b2"],
        inputs["Wfc"], inputs["bfc"],
    )
    if TS not in _CACHE:
        _CACHE[TS] = _build_program(TS)
    nc = _CACHE[TS]
    res = run_bass_kernel_spmd(nc, in_maps, list(range(N_CORES)))
    outs = [res.results[c]["out"].reshape(-1) for c in range(N_CORES)]
    return np.concatenate(outs).astype(np.float32)


# revision 46
# speedup vs baseline: 2.8938x; 2.0657x over previous
"""Trainium2 Bass kernel for nn_AffinityPredictor (2-layer GCN + mean-pool + FC).

Contract: kernel(**inputs) takes the FULL unsharded inputs (as produced by
reference.setup_inputs()) and returns the FULL [1024] output.

v2 design (8 NeuronCores, SPMD — one program, per-core data):
  * Graph-parallel sharding: core c owns graphs [128c, 128(c+1)) and the
    contiguous node range they span (batch is sorted); it owns all non-self
    edges whose dst lies in that range.  Self-loops are applied densely.
  * Nodes are padded to NPAD=12800 per core = NT=100 tiles of P=128; local id
    l = t*128 + p.  Feature tables are stored as bf16 pair rows (two nodes
    per 256-byte row) for the swdge bulk-gather (int16 indices, 256B elems).
    Each core's nodes split into half A (l < 6400) and half B so pair-row ids
    fit int16 (25600 rows per half) — the halves double as the two slabs of
    the per-group gather segments AND as the two AllGather chunks.
  * Per (dst-tile g, half s) the incident edges are packed into 128-slot
    tiles with pads only at the segment tail; the gather's runtime
    num_idxs_reg (per-core counts input) trims pad descriptors entirely.
  * The segment-sum is PE matmuls: per edge-tile, two matmuls (one per
    pair-half) against a [128, 128]-wide one-hot block that routes each slot
    to its dst column.  One-hots are GENERATED ON DEVICE (DVE is_equal of a
    per-slot code word against an iota row) instead of streamed from HBM.
  * GCN norm D^-1/2(A+I)D^-1/2 folds in as activation scales: tables carry
    dinv(src); L1's flush scale is dinv^2(dst) which bakes table2 rows
    dinv*(relu(conv1) @ W2) so L2 needs only dinv(dst); rank-1 sqrt(deg)
    bias matmuls keep b1/b2 exact under the scales.
  * L1's per-group transform (transpose - W2 - transpose) is interleaved
    with the group loop; the AllGather runs in two halves (after group 49
    and 99) so exchange overlaps the L1 tail.  Gathered Shared tables are
    bounced to regular DRAM (Shared-space gathers are slower).
  * Mean pooling via binary one-hot matmuls folded into the L2 loop; 1/count
    and the fc bias applied to the final [1, 128] result.
"""

import os
import numpy as np
import ml_dtypes

import concourse.tile as tile
from concourse import bass, bacc, mybir
from concourse.bass import broadcast_tensor_aps
from concourse.bass_utils import run_bass_kernel_spmd

# ---------------------------------------------------------------- constants
N_NODES = 100_000
NUM_GRAPHS = 1024
IN_DIM = 20
NODE_DIM = 64
N_CORES = 8
GPC = NUM_GRAPHS // N_CORES                  # graphs per core = 128
P = 128

NPAD = 12_800                                # padded nodes per core
NT = NPAD // P                               # node tiles (= groups) = 100
NTH = NT // 2                                # tiles per half = 50
HALF = NPAD // 2                             # nodes per half = 6400
QH = N_CORES * HALF // 2                     # pair rows per half-table = 25600
NSEG = NT * 2                                # gather segments (g, s)
WIN = 64                                     # dst window (one-hot block) size
NWIN = P // WIN                              # windows per group = 2

PAD_CODE = 300.0                             # one-hot code for pad slots

BF16 = mybir.dt.bfloat16
F32 = mybir.dt.float32
I16 = mybir.dt.int16
I32 = mybir.dt.int32

_CACHE = {}


# ================================================================ host prep
def _preprocess(x, edge_index, batch, W1, b1, W2, b2, Wfc, bfc):
    x = np.asarray(x, np.float32)
    edge_index = np.asarray(edge_index, np.int64)
    batch = np.asarray(batch, np.int64)
    n = N_NODES

    deg = (np.bincount(edge_index[1], minlength=n) + 1).astype(np.float32)
    dinv = (1.0 / np.sqrt(deg)).astype(np.float32)
    sdeg = np.sqrt(deg).astype(np.float32)

    gbound = np.searchsorted(batch, np.arange(0, NUM_GRAPHS + 1, GPC))
    n0s, n1s = gbound[:-1], gbound[1:]
    core_of = np.searchsorted(gbound[1:], np.arange(n), side="right")
    local_of = np.arange(n) - n0s[core_of]           # l = t*128 + p

    # ---- per-edge fields (dst-core partition, self-loops excluded)
    e_src, e_dst = edge_index[0], edge_index[1]
    ecore = core_of[e_dst]
    l_s = local_of[e_src]
    s_half = (l_s >= HALF).astype(np.int64)
    q_all = core_of[e_src] * (HALF // 2) + np.where(s_half == 0, l_s, l_s - HALF) // 2
    hf_all = (l_s % 2).astype(np.int64)

    cnts = np.zeros((N_CORES, NT, 2, 2), np.int64)               # [c, g, s, w]
    per_core = []
    for c in range(N_CORES):
        m = ecore == c
        ld = (e_dst[m] - n0s[c]).astype(np.int64)
        g = ld // P
        w = (ld % P) // WIN
        key = (g * 2 + s_half[m]) * 2 + w
        order = np.argsort(key, kind="stable")
        per_core.append((q_all[m][order], hf_all[m][order],
                         (ld % P)[order], key[order]))
        cnts[c] = np.bincount(key, minlength=NSEG * 2).reshape(NT, 2, 2)

    tw = np.ceil(cnts.max(axis=0) / P).astype(np.int64)          # [NT, 2, 2]
    ts = tw.sum(axis=2)                                          # [NT, 2]
    seg_tiles = ts.reshape(-1)                                   # [NSEG]
    seg_base = np.zeros(NSEG + 1, np.int64)
    seg_base[1:] = np.cumsum(seg_tiles)
    T_TOTAL = int(seg_base[-1])
    # slot base of each (g, s, w) sub-segment
    sub_base = np.zeros((NT, 2, 2), np.int64)
    sub_base[:, :, 0] = seg_base[:-1].reshape(NT, 2)
    sub_base[:, :, 1] = seg_base[:-1].reshape(NT, 2) + tw[:, :, 0]

    in_maps = []
    for c in range(N_CORES):
        q, hf, d, key = per_core[c]
        cnt = np.bincount(key, minlength=NSEG * 2)               # per (g,s,w)
        starts = np.zeros(NSEG * 2, np.int64)
        starts[1:] = np.cumsum(cnt)[:-1]
        pos = np.arange(len(q)) - starts[key]
        slot = sub_base.reshape(-1)[key] * P + pos               # global slot

        # w0 pads fetch row 0 (valid, zero one-hot); w1 tail pads trimmed
        idx_flat = np.full(T_TOTAL * P, -1, np.int16)
        wv_flat = np.full(T_TOTAL * P, PAD_CODE, np.float32)
        for g_ in range(NT):
            for s_ in range(2):
                b0 = sub_base[g_, s_, 0] * P
                idx_flat[b0:b0 + tw[g_, s_, 0] * P] = 0
        idx_flat[slot] = q.astype(np.int16)
        wv_flat[slot] = (hf * WIN + d % WIN).astype(np.float32)
        cnt2 = cnt.reshape(NT, 2, 2)
        gcnt = (tw[:, :, 0] * P + cnt2[:, :, 1]).astype(np.int32).reshape(-1)
        # dummy valid descriptor for empty segments with tiles>0
        empty = (gcnt == 0) & (seg_tiles > 0)
        for k in np.nonzero(empty)[0]:
            idx_flat[seg_base[k] * P] = 0
            gcnt[k] = 1
        if os.environ.get("PAD_FULL"):
            # probe mode: no runtime trimming — pads fetch row 0
            idx_flat[idx_flat < 0] = 0
            gcnt = (seg_tiles * P).astype(np.int32)

        # idx device layout: 16-partition wrap, element i at [i%16, i//16],
        # replicated to all 8 gpsimd cores' partition groups
        idx_dev = np.tile(np.ascontiguousarray(
            idx_flat.reshape(T_TOTAL * P // 16, 16).T), (8, 1))
        # wvals: slot i of tile t -> [i%128, t]
        wv_dev = np.ascontiguousarray(
            wv_flat.reshape(T_TOTAL, P).T).astype(ml_dtypes.bfloat16)

        # ---- per-node data, local layout (node l = 128 t + p)
        n_real = int(n1s[c] - n0s[c])
        l_arr = np.arange(NPAD)
        gl = np.minimum(n0s[c] + l_arr, n - 1)
        valid = l_arr < n_real
        dinv_l = np.where(valid, dinv[gl], 0.0).astype(np.float32)
        sdeg_l = np.where(valid, sdeg[gl], 0.0).astype(np.float32)
        dinvp = np.ascontiguousarray(dinv_l.reshape(NT, P).T)
        dinv2p = np.ascontiguousarray((dinv_l ** 2).reshape(NT, P).T)

        xs_l = np.zeros((NPAD, IN_DIM), np.float32)
        xs_l[valid] = dinv[gl[valid], None] * x[gl[valid]]
        # xr: [p, t*20+j] = xs_l[t*128+p, j]
        xr = np.ascontiguousarray(
            xs_l.reshape(NT, P, IN_DIM).transpose(1, 0, 2)).reshape(P, NT * IN_DIM)

        # ---- pooling one-hot (binary) + 1/count
        cnt_g = np.bincount((batch[n0s[c]:n1s[c]] - c * GPC).astype(np.int64),
                            minlength=GPC).astype(np.float32)
        invc = (1.0 / np.maximum(cnt_g, 1.0)).astype(np.float32).reshape(1, GPC)
        lg = (batch[gl] - c * GPC).astype(np.int64)
        poolh = np.zeros((NPAD, GPC), ml_dtypes.bfloat16)
        poolh[l_arr[valid], lg[valid]] = 1.0
        poolh = np.ascontiguousarray(
            poolh.reshape(NT, P, GPC).transpose(1, 0, 2)).reshape(P, NT * GPC)

        in_maps.append({
            "gidx": idx_dev, "wvals": wv_dev, "gcnt": gcnt.reshape(1, NSEG),
            "xr": xr.astype(ml_dtypes.bfloat16),
            "dinvp": dinvp, "dinv2p": dinv2p, "invc": invc, "poolh": poolh,
            "sdegrow": sdeg_l.reshape(1, NPAD).astype(ml_dtypes.bfloat16),
        })

    # ---- replicated tables: x*dinv in half-A/B pair-row order
    # half A rows: node (c, l<6400) at pair qa = c*3200 + l//2, col hf*64+j
    xs = dinv[:, None] * x                                        # [n, 20]
    xtab = np.zeros((2, QH, 2 * NODE_DIM), np.float32)
    nodes = np.arange(n)
    l_n = local_of[nodes]
    s_n = (l_n >= HALF).astype(np.int64)
    q_n = core_of[nodes] * (HALF // 2) + np.where(s_n == 0, l_n, l_n - HALF) // 2
    hf_n = l_n % 2
    for j in range(IN_DIM):
        xtab[s_n, q_n, hf_n * NODE_DIM + j] = xs[nodes, j]
    xtabA = xtab[0].astype(ml_dtypes.bfloat16)
    xtabB = xtab[1].astype(ml_dtypes.bfloat16)

    iota = np.tile(np.arange(256, dtype=np.float32), (P, 1)).astype(
        ml_dtypes.bfloat16)

    shared = {
        "xtabA": xtabA, "xtabB": xtabB, "iota": iota,
        "ident": np.eye(P, dtype=ml_dtypes.bfloat16),
        "w1": np.asarray(W1, np.float32).astype(ml_dtypes.bfloat16),
        "b1row": np.asarray(b1, np.float32).reshape(1, NODE_DIM)
            .astype(ml_dtypes.bfloat16),
        "b2row": np.asarray(b2, np.float32).reshape(1, NODE_DIM)
            .astype(ml_dtypes.bfloat16),
        "w2": np.asarray(W2, np.float32).astype(ml_dtypes.bfloat16),
        "wfc": np.asarray(Wfc, np.float32).astype(ml_dtypes.bfloat16),
        "bfc": np.full((1, GPC), np.float32(np.asarray(bfc).reshape(-1)[0])),
    }
    for m in in_maps:
        m.update(shared)
    return in_maps, tuple(tuple(tuple(x) for x in row) for row in tw.tolist())


# ============================================================= device program
def _build_null(TS):
    """Input-identical no-op program for differential timing."""
    tw = np.asarray(TS, np.int64)
    T_TOTAL = int(tw.sum())
    nc = bacc.Bacc(num_swdge_queues=4)
    nc.declare_dram_parameter("xtabA", [QH, 2 * NODE_DIM], BF16, isOutput=False)
    nc.declare_dram_parameter("xtabB", [QH, 2 * NODE_DIM], BF16, isOutput=False)
    nc.declare_dram_parameter("gidx", [P, T_TOTAL * 8], I16, isOutput=False)
    nc.declare_dram_parameter("wvals", [P, T_TOTAL], BF16, isOutput=False)
    nc.declare_dram_parameter("gcnt", [1, NSEG], I32, isOutput=False)
    nc.declare_dram_parameter("xr", [P, NT * IN_DIM], BF16, isOutput=False)
    nc.declare_dram_parameter("iota", [P, 256], BF16, isOutput=False)
    nc.declare_dram_parameter("ident", [P, P], BF16, isOutput=False)
    nc.declare_dram_parameter("w1", [IN_DIM, NODE_DIM], BF16, isOutput=False)
    nc.declare_dram_parameter("b1row", [1, NODE_DIM], BF16, isOutput=False)
    nc.declare_dram_parameter("b2row", [1, NODE_DIM], BF16, isOutput=False)
    nc.declare_dram_parameter("sdegrow", [1, NPAD], BF16, isOutput=False)
    nc.declare_dram_parameter("w2", [NODE_DIM, NODE_DIM], BF16, isOutput=False)
    nc.declare_dram_parameter("wfc", [NODE_DIM, 1], BF16, isOutput=False)
    nc.declare_dram_parameter("bfc", [1, GPC], F32, isOutput=False)
    nc.declare_dram_parameter("invc", [1, GPC], F32, isOutput=False)
    nc.declare_dram_parameter("dinvp", [P, NT], F32, isOutput=False)
    nc.declare_dram_parameter("dinv2p", [P, NT], F32, isOutput=False)
    nc.declare_dram_parameter("poolh", [P, NT * GPC], BF16, isOutput=False)
    out = nc.declare_dram_parameter("out", [1, GPC], F32, isOutput=True)
    with tile.TileContext(nc) as tc:
        with tc.tile_pool(name="stage", bufs=1) as stagep:
            zo = stagep.tile([1, GPC], F32, tag="osb")
            nc.vector.memset(zo[:], 0.0)
            nc.sync.dma_start(out=out[:], in_=zo[:])
    nc.compile()
    return nc


def _build_program(TS, stages=5, repeat=1, parts='all', nq=4,
                   gt_bufs=5, oh_bufs=2, debug=False, xg='after',
                   prep_trig=0):
    if stages == 0:
        return _build_null(TS)
    tw = np.asarray(TS, np.int64)                    # [NT, 2, 2]
    ts = tw.sum(axis=2)                              # [NT, 2]
    seg_tiles = ts.reshape(-1)
    seg_base = np.zeros(NSEG + 1, np.int64)
    seg_base[1:] = np.cumsum(seg_tiles)
    T_TOTAL = int(seg_base[-1])
    TPG = ts.sum(1)                                  # tiles per group
    TPGMAX = int(TPG.max())
    grp_base = np.zeros(NT + 1, np.int64)
    grp_base[1:] = np.cumsum(TPG)
    # window of each tile within its group (slab-a w0 tiles, a w1, b w0, b w1)
    wlist_g = [
        [w for s in range(2) for w in ([0] * int(tw[g, s, 0]) +
                                       [1] * int(tw[g, s, 1]))]
        for g in range(NT)
    ]

    nc = bacc.Bacc(num_swdge_queues=nq)
    xtabA = nc.declare_dram_parameter("xtabA", [QH, 2 * NODE_DIM], BF16, isOutput=False)
    xtabB = nc.declare_dram_parameter("xtabB", [QH, 2 * NODE_DIM], BF16, isOutput=False)
    gidx = nc.declare_dram_parameter("gidx", [P, T_TOTAL * 8], I16, isOutput=False)
    wvals = nc.declare_dram_parameter("wvals", [P, T_TOTAL], BF16, isOutput=False)
    gcnt = nc.declare_dram_parameter("gcnt", [1, NSEG], I32, isOutput=False)
    xr = nc.declare_dram_parameter("xr", [P, NT * IN_DIM], BF16, isOutput=False)
    iota = nc.declare_dram_parameter("iota", [P, 256], BF16, isOutput=False)
    ident = nc.declare_dram_parameter("ident", [P, P], BF16, isOutput=False)
    w1 = nc.declare_dram_parameter("w1", [IN_DIM, NODE_DIM], BF16, isOutput=False)
    b1row = nc.declare_dram_parameter("b1row", [1, NODE_DIM], BF16, isOutput=False)
    b2row = nc.declare_dram_parameter("b2row", [1, NODE_DIM], BF16, isOutput=False)
    sdegrow = nc.declare_dram_parameter("sdegrow", [1, NPAD], BF16, isOutput=False)
    w2 = nc.declare_dram_parameter("w2", [NODE_DIM, NODE_DIM], BF16, isOutput=False)
    wfc = nc.declare_dram_parameter("wfc", [NODE_DIM, 1], BF16, isOutput=False)
    bfc = nc.declare_dram_parameter("bfc", [1, GPC], F32, isOutput=False)
    invc = nc.declare_dram_parameter("invc", [1, GPC], F32, isOutput=False)
    dinvp = nc.declare_dram_parameter("dinvp", [P, NT], F32, isOutput=False)
    dinv2p = nc.declare_dram_parameter("dinv2p", [P, NT], F32, isOutput=False)
    poolh = nc.declare_dram_parameter("poolh", [P, NT * GPC], BF16, isOutput=False)
    out = nc.declare_dram_parameter("out", [1, GPC], F32, isOutput=True)
    if debug:
        dbg_gt = nc.declare_dram_parameter("dbg_gt", [P, TPGMAX * P], BF16, isOutput=True)
        dbg_oh = nc.declare_dram_parameter("dbg_oh", [P, TPGMAX * 256], BF16,
                                           isOutput=True)  # oversized scratch
        dbg_agg = nc.declare_dram_parameter("dbg_agg", [NODE_DIM, P], BF16, isOutput=True)
        dbg_h1w = nc.declare_dram_parameter("dbg_h1w", [P, NT * NODE_DIM], BF16, isOutput=True)
        dbg_t2 = nc.declare_dram_parameter("dbg_t2", [P, N_CORES * HALF * NODE_DIM // P], BF16, isOutput=True)
        dbg_pool = nc.declare_dram_parameter("dbg_pool", [NODE_DIM, GPC], BF16, isOutput=True)
        dbg_hg = nc.declare_dram_parameter("dbg_hg", [P, NODE_DIM], BF16, isOutput=True)
        dbg_hwT = nc.declare_dram_parameter("dbg_hwT", [NODE_DIM, P], BF16, isOutput=True)
        dbg_a = nc.declare_dram_parameter("dbg_a", [P, NT * NODE_DIM], BF16, isOutput=True)
        dbg_b = nc.declare_dram_parameter("dbg_b", [P, NT * NODE_DIM], BF16, isOutput=True)

    h1sA = nc.dram_tensor("h1sA", [N_CORES * HALF // N_CORES, NODE_DIM], BF16)
    h1sB = nc.dram_tensor("h1sB", [HALF, NODE_DIM], BF16)
    t2a = nc.dram_tensor("t2a", [N_CORES * HALF, NODE_DIM], BF16, addr_space="Shared")
    t2b = nc.dram_tensor("t2b", [N_CORES * HALF, NODE_DIM], BF16, addr_space="Shared")
    t2al = nc.dram_tensor("t2al", [N_CORES * HALF, NODE_DIM], BF16)
    t2bl = nc.dram_tensor("t2bl", [N_CORES * HALF, NODE_DIM], BF16)

    xtA_pair = xtabA[:]                                        # [25600, 128]
    xtB_pair = xtabB[:]
    t2al_pair = t2al[:].rearrange("(q two) d -> q (two d)", two=2)
    t2bl_pair = t2bl[:].rearrange("(q two) d -> q (two d)", two=2)

    with tile.TileContext(nc) as tc:
        with (
            tc.tile_pool(name="const", bufs=1) as constp,
            tc.tile_pool(name="stage", bufs=2) as stagep,
            tc.tile_pool(name="gat", bufs=gt_bufs) as gatp,
            tc.tile_pool(name="ohp", bufs=oh_bufs) as ohp,
            tc.tile_pool(name="hsb", bufs=1) as hsbp,
            tc.tile_pool(name="hg", bufs=2) as hgp,
            tc.tile_pool(name="php", bufs=2) as php,
            tc.tile_pool(name="psE", bufs=2, space="PSUM") as psE,
            tc.tile_pool(name="psD", bufs=2, space="PSUM") as psD,
            tc.tile_pool(name="psT", bufs=1, space="PSUM") as psT,
            tc.tile_pool(name="psC", bufs=1, space="PSUM") as psC,
        ):
            # ---------------- constants
            w1_sb = constp.tile([IN_DIM, NODE_DIM], BF16)
            b1row_sb = constp.tile([1, NODE_DIM], BF16)
            b2row_sb = constp.tile([1, NODE_DIM], BF16)
            sdeg_sb = constp.tile([1, NPAD], BF16)
            w2_sb = constp.tile([NODE_DIM, NODE_DIM], BF16)
            wfc_sb = constp.tile([NODE_DIM, 1], BF16)
            bfc_sb = constp.tile([1, GPC], F32)
            invc_sb = constp.tile([1, GPC], F32)
            dinv_sb = constp.tile([P, NT], F32)
            dinv2_sb = constp.tile([P, NT], F32)
            id_sb = constp.tile([P, P], BF16)
            io_sb = constp.tile([P, 256], BF16)
            xr_sb = constp.tile([P, NT * IN_DIM], BF16)
            wv_sb = constp.tile([P, T_TOTAL], BF16)
            ix_sb = constp.tile([P, T_TOTAL * 8], I16)
            cnt_sb = constp.tile([1, NSEG], I32)
            for dst_t, src_t in ((w1_sb, w1), (b1row_sb, b1row),
                                 (b2row_sb, b2row), (sdeg_sb, sdegrow),
                                 (w2_sb, w2), (wfc_sb, wfc), (bfc_sb, bfc),
                                 (invc_sb, invc), (dinv_sb, dinvp),
                                 (dinv2_sb, dinv2p), (id_sb, ident),
                                 (io_sb, iota), (xr_sb, xr), (cnt_sb, gcnt)):
                nc.sync.dma_start(out=dst_t[:], in_=src_t[:])
            nc.scalar.dma_start(out=wv_sb[:], in_=wvals[:])
            nc.scalar.dma_start(out=ix_sb[:], in_=gidx[:])

            h1w = hsbp.tile([P, NT * NODE_DIM], BF16, tag="h1w")
            pool_ps = psC.tile([NODE_DIM, GPC], F32, tag="pps")
            PHC = 25
            ph_sb = None
            cnt_reg = nc.gpsimd.alloc_register("gcnt_reg")
            gsem = [nc.alloc_semaphore(f"gsem{q}") for q in range(nq)]

            static_gt = None
            if parts == 'mm_only':
                static_gt = gatp.tile([P, TPGMAX * P], BF16, tag="gt")
                nc.vector.memset(static_gt[:], 0.0)

            # memset the gather buffers once so stale SBUF can't be NaN
            if parts != 'mm_only':
                for _ in range(gt_bufs):
                    z = gatp.tile([P, TPGMAX * P], BF16, tag="gt")
                    nc.vector.memset(z[:], 0.0)

            layer_list = [l for l in (1, 2)] * repeat
            do_mm = parts in ('all', 'mm_only')
            do_gather = parts in ('all', 'gather_only')

            for layer in layer_list:
                tabs = (xtA_pair, xtB_pair) if layer == 1 else (t2al_pair, t2bl_pair)
                scale_sb = dinv2_sb if layer == 1 else dinv_sb
                for g in range(NT):
                    gb = int(grp_base[g])
                    tpg = int(TPG[g])
                    if tpg == 0:
                        continue
                    # ---- one-hot generation for this group's tiles
                    oh = None
                    if do_mm:
                        oh = ohp.tile([P, TPGMAX * 2 * WIN], BF16, tag="oh")
                        oh3 = oh[:, :tpg * 2 * WIN].rearrange(
                            "p (t j) -> p t j", j=2 * WIN)
                        b0, b1b = broadcast_tensor_aps(
                            wv_sb[:, gb:gb + tpg].rearrange("p (t o) -> p t o", o=1),
                            io_sb[:, :2 * WIN].rearrange("p (o j) -> p o j", o=1))
                        nc.vector.tensor_tensor(out=oh3, in0=b0, in1=b1b,
                                                op=mybir.AluOpType.is_equal)
                    # ---- gathers (one per half-segment, runtime counts,
                    #      prepare+trigger so desc-gen overlaps the drain)
                    if parts == 'mm_only':
                        gt = static_gt
                    else:
                        gt = gatp.tile([P, TPGMAX * P], BF16, tag="gt")
                    if do_gather:
                        t0 = 0
                        qn = g % nq
                        prepped = False
                        for s in range(2):
                            nts = int(ts[g, s])
                            if nts == 0:
                                continue
                            k = g * 2 + s
                            sb = int(seg_base[k])
                            nc.gpsimd.reg_load(cnt_reg, cnt_sb[0:1, k:k + 1])
                            gkw = (dict(prepare_only=True, sem=gsem[qn])
                                   if prep_trig else {})
                            nc.gpsimd.dma_gather(
                                out_ap=gt[:, t0 * P:(t0 + nts) * P].rearrange(
                                    "p (t r) -> p t r", r=P),
                                in_ap=tabs[s][:],
                                idxs_ap=ix_sb[:, sb * 8:(sb + nts) * 8],
                                num_idxs=nts * P,
                                num_idxs_reg=cnt_reg,
                                elem_size=2 * NODE_DIM,
                                single_packet=False,
                                queue_num=qn,
                                **gkw,
                            )
                            prepped = True
                            t0 += nts
                        if prepped and prep_trig:
                            nc.gpsimd.trigger_dma(count=None, queue_num=qn)
                    if not do_mm:
                        continue
                    if debug and layer == 1 and g == 0:
                        nc.sync.dma_start(out=dbg_gt[:], in_=gt[:])
                        nc.sync.dma_start(out=dbg_oh[:, :TPGMAX * 2 * WIN],
                                          in_=oh[:])
                    # ---- aggregation matmuls, bracketed per dst window
                    fd = IN_DIM if layer == 1 else NODE_DIM
                    ps = psE.tile([NODE_DIM, P], F32, tag="agg")
                    if layer == 1:
                        self_lhsT = xr_sb[:, g * IN_DIM:(g + 1) * IN_DIM]
                    else:
                        self_lhsT = h1w[:, g * NODE_DIM:(g + 1) * NODE_DIM]
                    wl = wlist_g[g]
                    for w in range(NWIN):
                        # self-loop term opens the window's accumulation
                        mms = [(self_lhsT, id_sb[:, w * WIN:(w + 1) * WIN])]
                        for t in range(tpg):
                            if wl[t] != w:
                                continue
                            for hf in range(2):
                                mms.append((gt[:, t * P + hf * NODE_DIM:
                                               t * P + hf * NODE_DIM + fd],
                                            oh[:, t * 2 * WIN + hf * WIN:
                                               t * 2 * WIN + (hf + 1) * WIN]))
                        for i, (lhsT, rhs) in enumerate(mms):
                            nc.tensor.matmul(
                                out=ps[0:fd, w * WIN:(w + 1) * WIN],
                                lhsT=lhsT, rhs=rhs,
                                start=(i == 0), stop=(i == len(mms) - 1))
                    # ---- flush
                    aggT = stagep.tile([NODE_DIM, P], BF16, tag="aggT")
                    nc.vector.tensor_copy(out=aggT[0:fd, :], in_=ps[0:fd, :])
                    if debug and layer == 1 and g == 0:
                        nc.sync.dma_start(out=dbg_agg[0:fd, :], in_=aggT[0:fd, :])
                    ps2 = psD.tile([P, NODE_DIM], F32, tag="ps2")
                    if layer == 1:
                        nc.tensor.matmul(out=ps2[:], lhsT=aggT[0:fd, :], rhs=w1_sb[:],
                                         start=True, stop=False)
                        brow = b1row_sb
                    else:
                        nc.tensor.matmul(out=ps2[:], lhsT=aggT[0:fd, :],
                                         rhs=id_sb[:NODE_DIM, :NODE_DIM],
                                         start=True, stop=False)
                        brow = b2row_sb
                    nc.tensor.matmul(out=ps2[:], lhsT=sdeg_sb[:, g * P:(g + 1) * P],
                                     rhs=brow[:], start=False, stop=True)
                    hg = hgp.tile([P, NODE_DIM], BF16, tag="hg")
                    nc.scalar.activation(
                        out=hg[:], in_=ps2[:],
                        func=mybir.ActivationFunctionType.Relu,
                        scale=scale_sb[:, g:g + 1])
                    if debug and layer == 1 and g == 0:
                        nc.sync.dma_start(out=dbg_hg[:], in_=hg[:])

                    if layer == 1:
                        # transform: h1w[:, g] = (relu(...) @ W2) (node-major)
                        psT1 = psT.tile([NODE_DIM, P], BF16, tag="tr")
                        nc.tensor.transpose(out=psT1[:], in_=hg[:], identity=id_sb[:])
                        hT = stagep.tile([NODE_DIM, P], BF16, tag="hT")
                        nc.vector.tensor_copy(out=hT[:], in_=psT1[:])
                        psT2 = psT.tile([NODE_DIM, P], F32, tag="tr")
                        nc.tensor.matmul(out=psT2[:], lhsT=w2_sb[:], rhs=hT[:],
                                         start=True, stop=True)
                        hwT = stagep.tile([NODE_DIM, P], BF16, tag="hwT")
                        nc.vector.tensor_copy(out=hwT[:], in_=psT2[:])
                        if debug and g == 0:
                            nc.sync.dma_start(out=dbg_hwT[:], in_=hwT[:])
                        psT3 = psT.tile([P, NODE_DIM], BF16, tag="tr2")
                        nc.tensor.transpose(out=psT3[:], in_=hwT[:],
                                            identity=id_sb[:NODE_DIM, :NODE_DIM])
                        nc.scalar.copy(out=h1w[:, g * NODE_DIM:(g + 1) * NODE_DIM],
                                       in_=psT3[:])
                        if debug and g == 0:
                            nc.sync.dma_start(out=dbg_hg[:],
                                              in_=h1w[:, 0:NODE_DIM])
                    else:
                        # pooling accumulate
                        if g % PHC == 0:
                            ph_sb = php.tile([P, PHC * GPC], BF16, tag="ph")
                            nc.scalar.dma_start(
                                out=ph_sb[:],
                                in_=poolh[:, g * GPC:(g + PHC) * GPC])
                        nc.tensor.matmul(
                            out=pool_ps[:],
                            lhsT=hg[:],
                            rhs=ph_sb[:, (g % PHC) * GPC:(g % PHC + 1) * GPC],
                            start=(g == 0), stop=(g == NT - 1))

                    # ---- half-table exchange, interleaved with L1 tail
                    if (layer == 1 and g in (NTH - 1, NT - 1) and parts == 'all'
                            and xg == 'inline'):
                        first = g == NTH - 1
                        hsl = h1sA if first else h1sB
                        t2 = t2a if first else t2b
                        t2l = t2al if first else t2bl
                        c0 = 0 if first else NTH * NODE_DIM
                        if debug and first:
                            nc.sync.dma_start(out=dbg_a[:], in_=h1w[:])
                        if debug and not first:
                            nc.sync.dma_start(out=dbg_oh[:, :NT * NODE_DIM],
                                              in_=h1w[:])
                        nc.sync.dma_start(
                            out=hsl[:].rearrange("(t p) d -> p t d", p=P),
                            in_=h1w[:, c0:c0 + NTH * NODE_DIM].rearrange(
                                "p (t d) -> p t d", d=NODE_DIM))
                        nc.gpsimd.collective_compute(
                            "AllGather",
                            mybir.AluOpType.bypass,
                            replica_groups=[list(range(N_CORES))],
                            ins=[hsl[:]],
                            outs=[t2[:]],
                        )
                        nc.sync.dma_start(
                            out=t2l[:].rearrange("(p r) d -> p (r d)", p=P),
                            in_=t2[:].rearrange("(p r) d -> p (r d)", p=P))
                        if debug and first:
                            nc.sync.dma_start(out=dbg_b[:], in_=h1w[:])
                        if not first:
                            tc.strict_bb_all_engine_barrier()
                            if debug:
                                nc.sync.dma_start(out=dbg_h1w[:], in_=h1w[:])
                                nc.sync.dma_start(
                                    out=dbg_t2[:],
                                    in_=t2al[:].rearrange("(p r) d -> p (r d)", p=P))
                                nc.sync.dma_start(
                                    out=dbg_gt[:, :NTH * NODE_DIM],
                                    in_=h1sA[:].rearrange("(t p) d -> p t d", p=P))

                if layer == 1 and parts == 'all' and xg == 'after':
                    for first in (True, False):
                        hsl = h1sA if first else h1sB
                        t2 = t2a if first else t2b
                        c0 = 0 if first else NTH * NODE_DIM
                        nc.sync.dma_start(
                            out=hsl[:].rearrange("(t p) d -> p t d", p=P),
                            in_=h1w[:, c0:c0 + NTH * NODE_DIM].rearrange(
                                "p (t d) -> p t d", d=NODE_DIM))
                        nc.gpsimd.collective_compute(
                            "AllGather",
                            mybir.AluOpType.bypass,
                            replica_groups=[list(range(N_CORES))],
                            ins=[hsl[:]],
                            outs=[t2[:]],
                        )
                    for t2, t2l in ((t2a, t2al), (t2b, t2bl)):
                        nc.sync.dma_start(
                            out=t2l[:].rearrange("(p r) d -> p (r d)", p=P),
                            in_=t2[:].rearrange("(p r) d -> p (r d)", p=P))
                    tc.strict_bb_all_engine_barrier()
                    if debug:
                        nc.sync.dma_start(out=dbg_h1w[:], in_=h1w[:])
                        nc.sync.dma_start(
                            out=dbg_t2[:],
                            in_=t2al[:].rearrange("(p r) d -> p (r d)", p=P))
                if layer == 1 and parts != 'all':
                    # no exchange in attribution modes: fill h1w deterministically
                    nc.vector.memset(h1w[:], 0.0)
                    tc.strict_bb_all_engine_barrier()

            # ---------------- mean pool + fc
            if parts != 'all':
                zo = stagep.tile([1, GPC], F32, tag="osb")
                nc.vector.memset(zo[:], 0.0)
                nc.sync.dma_start(out=out[:], in_=zo[:])
            else:
                pool_sb = stagep.tile([NODE_DIM, GPC], BF16, tag="pool")
                nc.vector.tensor_copy(out=pool_sb[:], in_=pool_ps[:])
                if debug:
                    nc.sync.dma_start(out=dbg_pool[:], in_=pool_sb[:])
                fc_ps = psC.tile([1, GPC], F32, tag="fc")
                nc.tensor.matmul(out=fc_ps[:], lhsT=wfc_sb[:], rhs=pool_sb[:],
                                 start=True, stop=True)
                out_sb = stagep.tile([1, GPC], F32, tag="osb")
                nc.vector.tensor_tensor(out=out_sb[:], in0=fc_ps[:], in1=invc_sb[:],
                                        op=mybir.AluOpType.mult)
                nc.vector.tensor_tensor(out=out_sb[:], in0=out_sb[:], in1=bfc_sb[:],
                                        op=mybir.AluOpType.add)
                nc.sync.dma_start(out=out[:], in_=out_sb[:])

    nc.compile()
    return nc


# ================================================================== kernel
def kernel(**inputs) -> np.ndarray:
    in_maps, TS = _preprocess(
        inputs["x"], inputs["edge_index"], inputs["batch"],
        inputs["W1"], inputs["b1"], inputs["W2"], inputs["b2"],
        inputs["Wfc"], inputs["bfc"],
    )
    if TS not in _CACHE:
        _CACHE[TS] = _build_program(TS)
    nc = _CACHE[TS]
    res = run_bass_kernel_spmd(nc, in_maps, list(range(N_CORES)))
    outs = [res.results[c]["out"].reshape(-1) for c in range(N_CORES)]
    return np.concatenate(outs).astype(np.float32)
